# revision 1
# baseline (speedup 1.0000x reference)
"""Trainium2 kernel for nn_Non_Local_Sparse_Network (sparse_attention).

Device (8 NeuronCores, SPMD): the compute-heavy pipeline - all four conv
resblocks (fp32r matmuls, tap-paired), the LSH hashing rv = f1f2 @ [rot|-rot]
(fp32 matmuls) and the per-hash argmax bucket codes (DVE max/max_index).
Host: data routing + the small O(N) counting-sort bookkeeping and the final
rank-1 combine (the attention algebra collapses: q/z/r are built from only two
distinct rows because mod_indices = (indices % 2) * L).
"""
import numpy as np

"""Patch TileContext._drain_and_barrier: this walrus build only accepts one
sync-wait on an SP Drain, so split the tail drain's waits across a chain of
single-wait drains."""
import bass_rust
import concourse.tile as _tile
from concourse.vector_clock import ScopedClock


def _drain_and_barrier_split(self, tick_clock, wait_clock):
    drain_inst = self.nc.sync.drain()
    wait_clock.add_sem_waits(
        drain_inst.ins, ScopedClock({None: tick_clock.global_clock})
    )
    si = drain_inst.ins.sync_info
    waits = list(si.on_wait)
    if len(waits) > 1:
        drain_inst.ins.sync_info = bass_rust.SyncInfo(
            on_wait=[waits[0]], on_update=list(si.on_update)
        )
        for w in waits[1:]:
            extra = self.nc.sync.drain()
            extra.ins.sync_info = bass_rust.SyncInfo(on_wait=[w], on_update=[])

    self.nc.all_engine_barrier()
    assert self.sems is not None
    popped = self.nc._tile_sem_poison_stack.pop()
    assert popped is self._sem_poison
    self.nc.clear_and_free_semaphores(list(self.sems.allocated().values()))
    self.nc.all_engine_barrier()


_tile.TileContext._drain_and_barrier = _drain_and_barrier_split


def legalize_single_wait(nc):
    """This walrus build allows at most one sync-wait per instruction.
    For any instruction carrying k>1 waits, hoist k-1 of them onto fresh
    same-engine NOPs inserted immediately before it (same-engine program
    order makes this semantically identical)."""
    import concourse.mybir as mybir

    def make_nop(engine_type):
        eng = nc.engines[engine_type]
        binst = eng.nop()
        ins = binst.ins
        # eng.nop() appended to the current bb; pull it back out
        for fn in nc.m.functions:
            for bb in fn.blocks:
                il = bb.instructions
                if il and il[-1] is ins:
                    del il[-1]
                    return ins
        raise RuntimeError("fresh nop not found at tail of any bb")

    n_fixed = 0
    for fn in nc.m.functions:
        for bb in fn.blocks:
            il = bb.instructions
            i = 0
            while i < len(il):
                inst = il[i]
                try:
                    si = inst.sync_info
                except Exception:
                    si = None
                if si is None:
                    i += 1
                    continue
                waits = list(si.on_wait)
                if len(waits) > 1:
                    for w in waits[:-1]:
                        nop = make_nop(inst.engine)
                        nop.sync_info = bass_rust.SyncInfo(on_wait=[w], on_update=[])
                        il.insert(i, nop)
                        i += 1
                    inst.sync_info = bass_rust.SyncInfo(
                        on_wait=[waits[-1]], on_update=list(si.on_update)
                    )
                    n_fixed += 1
                i += 1
    return n_fixed


"""Launch 1: per-core = one a-conv half + one m-conv half + rv/argmax codes."""
import numpy as np
import concourse.bass as bass
import concourse.mybir as mybir
import concourse.tile as tile
from contextlib import ExitStack

F32 = mybir.dt.float32
F32R = mybir.dt.float32r
U32 = mybir.dt.uint32
PROW = 102
NROWS = 55
NFLAT = NROWS * PROW  # 5610
AF = mybir.ActivationFunctionType


def build_l1():
    nc = bass.Bass("TRN2", target_bir_lowering=False, debug=False, num_devices=8)
    xA = nc.dram_tensor("xA", [64, NFLAT], F32, kind="ExternalInput")
    xM = nc.dram_tensor("xM", [64, NFLAT], F32, kind="ExternalInput")
    wa1p = nc.dram_tensor("wa1p", [3, 128, 64], F32, kind="ExternalInput")
    wa1s = nc.dram_tensor("wa1s", [3, 64, 64], F32, kind="ExternalInput")
    wa2p = nc.dram_tensor("wa2p", [3, 128, 64], F32, kind="ExternalInput")
    wa2s = nc.dram_tensor("wa2s", [3, 64, 64], F32, kind="ExternalInput")
    ida = nc.dram_tensor("ida", [64, 64], F32, kind="ExternalInput")
    wm1p = nc.dram_tensor("wm1p", [3, 128, 16], F32, kind="ExternalInput")
    wm1s = nc.dram_tensor("wm1s", [3, 64, 16], F32, kind="ExternalInput")
    wm2p = nc.dram_tensor("wm2p", [3, 64, 16], F32, kind="ExternalInput")
    wm2s = nc.dram_tensor("wm2s", [3, 16, 16], F32, kind="ExternalInput")
    wskip = nc.dram_tensor("wskip", [64, 16], F32, kind="ExternalInput")
    rotpm = nc.dram_tensor("rotpm", [16, 512], F32, kind="ExternalInput")
    hmask = nc.dram_tensor("hmask", [128, 2], F32, kind="ExternalInput")

    aY = nc.dram_tensor("aY", [64, 5000], F32, kind="ExternalOutput")
    fY = nc.dram_tensor("fY", [16, 5000], F32, kind="ExternalOutput")
    codesU = nc.dram_tensor("codesU", [128, 1280], U32, kind="ExternalOutput")

    with tile.TileContext(nc) as tc, ExitStack() as ctx:
        const = ctx.enter_context(tc.tile_pool(name="const", bufs=1))
        big = ctx.enter_context(tc.tile_pool(name="big", bufs=1))
        work = ctx.enter_context(tc.tile_pool(name="work", bufs=3))
        psum = ctx.enter_context(tc.tile_pool(name="psum", bufs=2, space="PSUM"))
        pidx = ctx.enter_context(tc.tile_pool(name="pidx", bufs=2, space="PSUM"))

        def load_rounded(src, tag, dt):
            t32 = big.tile([64, NFLAT], F32, tag="ld32")
            nc.sync.dma_start(out=t32[:], in_=src[:])
            tr = big.tile([128, NFLAT], dt, tag=tag)
            nc.scalar.copy(tr[0:64, :], t32[:])
            nc.scalar.copy(tr[64:128, 0:NFLAT - PROW], t32[:, PROW:NFLAT])
            return tr

        xaR = load_rounded(xA, "xaR", F32R)
        xmR = load_rounded(xM, "xmR", F32)

        def load_w(src, k, cout, tag, dt=F32R):
            t = work.tile([k, cout], F32, tag="wld")
            nc.sync.dma_start(out=t[:], in_=src)
            tr = const.tile([k, cout], dt, tag=tag)
            nc.vector.tensor_copy(tr[:], t[:])
            return tr

        wa1 = [load_w(wa1p[i], 128, 64, f"wa1p{i}") for i in range(3)] + \
              [load_w(wa1s[i], 64, 64, f"wa1s{i}") for i in range(3)]
        wa2 = [load_w(wa2p[i], 128, 64, f"wa2p{i}") for i in range(3)] + \
              [load_w(wa2s[i], 64, 64, f"wa2s{i}") for i in range(3)]
        idaR = load_w(ida[:, :], 64, 64, "ida")
        wm1 = [load_w(wm1p[i], 128, 16, f"wm1p{i}", F32) for i in range(3)] + \
              [load_w(wm1s[i], 64, 16, f"wm1s{i}", F32) for i in range(3)]
        wm2 = [load_w(wm2p[i], 64, 16, f"wm2p{i}", F32) for i in range(3)] + \
              [load_w(wm2s[i], 16, 16, f"wm2s{i}", F32) for i in range(3)]
        wskipR = load_w(wskip[:, :], 64, 16, "wskip", F32)
        rot_t = const.tile([16, 512], F32)
        nc.sync.dma_start(out=rot_t[:], in_=rotpm[:])
        hm = const.tile([128, 2], F32, tag="hm")
        nc.sync.dma_start(out=hm[:], in_=hmask[:])

        def mask_h1(h1r, cout):
            # zero conv1 rows at image row -1 (u=0, half0) / 100 (u=51, half1)
            AL = mybir.AluOpType
            nc.vector.tensor_scalar(out=h1r[0:cout, 0:PROW],
                                    in0=h1r[0:cout, 0:PROW],
                                    scalar1=hm[0:cout, 0:1], scalar2=None, op0=AL.mult)
            nc.vector.tensor_scalar(out=h1r[0:cout, 51 * PROW:52 * PROW],
                                    in0=h1r[0:cout, 51 * PROW:52 * PROW],
                                    scalar1=hm[0:cout, 1:2], scalar2=None, op0=AL.mult)
            p2 = 64 if cout == 64 else 32
            nc.vector.tensor_scalar(out=h1r[p2:p2 + cout, 50 * PROW:51 * PROW],
                                    in0=h1r[p2:p2 + cout, 50 * PROW:51 * PROW],
                                    scalar1=hm[0:cout, 1:2], scalar2=None, op0=AL.mult)

        def r3(ap, nrowstile):
            return ap.rearrange("p (r c) -> p r c", c=PROW)

        # (drow, dcol, K) per matmul: 3 paired (ky=0&1) + 3 single (ky=2)
        def offs(cin):
            return [(0, kx, 2 * cin) for kx in range(3)] + \
                   [(2, kx, cin) for kx in range(3)]

        def conv1(xr, wts, om, cout, h1r, poff):
            x3 = r3(xr, NROWS)
            for j in range(11):
                y0 = 5 * j
                nrow = min(5, 52 - y0)
                n = nrow * 100
                pfull = psum.tile([64, 500], F32, tag="pconv")
                p = pfull[0:cout, :]
                for i, (dr, dc, k) in enumerate(om):
                    rhs = x3[0:k, y0 + dr:y0 + dr + nrow, dc:dc + 100]
                    nc.tensor.matmul(p[:, 0:n], wts[i][:], rhs,
                                     start=(i == 0), stop=(i == len(om) - 1))
                ps = p[:, 0:n].rearrange("p (r c) -> p r c", c=100)
                h3a = r3(h1r[0:cout, :], 52)
                nc.scalar.activation(h3a[:, y0:y0 + nrow, 1:101], ps, AF.Relu)
                h3b = r3(h1r[poff:poff + cout, :], 52)
                if j == 0:
                    ps1 = p[:, 100:n].rearrange("p (r c) -> p r c", c=100)
                    nc.scalar.activation(h3b[:, 0:nrow - 1, 1:101], ps1, AF.Relu)
                else:
                    nc.scalar.activation(h3b[:, y0 - 1:y0 - 1 + nrow, 1:101], ps, AF.Relu)

        def conv2(h1r, wtom, cout, elh, ext, k2, outdst):
            wts, om = wtom
            h3 = r3(h1r, 52)
            e3 = r3(ext, NROWS)
            for j in range(10):
                z0 = 5 * j
                pfull = psum.tile([64, 500], F32, tag="pconv")
                p = pfull[0:cout, :]
                for i, (dr, dc, k) in enumerate(om):
                    rhs = h3[0:k, z0 + dr:z0 + dr + 5, dc:dc + 100]
                    nc.tensor.matmul(p[:], wts[i][:], rhs, start=(i == 0), stop=False)
                rhs = e3[0:k2, z0 + 2:z0 + 7, 1:101]
                nc.tensor.matmul(p[:], elh[:], rhs, start=False, stop=True)
                nc.scalar.copy(outdst[:, 500 * j:500 * j + 500], p[:])

        h1aR = big.tile([128, 52 * PROW], F32R)
        nc.vector.memset(h1aR[:].bitcast(F32), 0.0)
        h1mR = big.tile([64, 52 * PROW], F32)
        nc.vector.memset(h1mR[:], 0.0)
        aYt = big.tile([64, 5000], F32)
        fYt = big.tile([16, 5000], F32)

        a2om = (wa2, offs(64))
        m2om = (wm2, [(0, kx, 64) for kx in range(3)] + [(2, kx, 16) for kx in range(3)])
        conv1(xaR, wa1, offs(64), 64, h1aR, 64)
        mask_h1(h1aR, 64)
        conv2(h1aR, a2om, 64, idaR, xaR, 64, aYt)
        conv1(xmR, wm1, offs(64), 16, h1mR, 32)
        mask_h1(h1mR, 16)
        conv2(h1mR, m2om, 16, wskipR, xmR, 64, fYt)

        nc.sync.dma_start(out=aY[:], in_=aYt[:])
        nc.sync.dma_start(out=fY[:], in_=fYt[:])

        codesT = big.tile([128, 1280], U32)
        nc.vector.memset(codesT[:], 0)
        for blk in range(40):
            m = min(128, 5000 - blk * 128)
            pr = pidx.tile([128, 512], F32, tag="rv")
            nc.tensor.matmul(pr[0:m, :], fYt[:, blk * 128:blk * 128 + m],
                             rot_t[:], start=True, stop=True)
            rvsb = work.tile([128, 512], F32, tag="rvsb")
            nc.vector.tensor_copy(rvsb[0:m, :], pr[0:m, :])
            for h in range(4):
                mx = work.tile([128, 8], F32, tag="mx")
                nc.vector.max(mx[0:m, :], rvsb[0:m, h * 128:(h + 1) * 128])
                nc.vector.max_index(
                    codesT[0:m, (blk * 4 + h) * 8:(blk * 4 + h) * 8 + 8],
                    mx[0:m, :], rvsb[0:m, h * 128:(h + 1) * 128])
        nc.sync.dma_start(out=codesU[:], in_=codesT[:])

    legalize_single_wait(nc)
    return nc


# ---- host-side input prep (inlined) ----
class cm:
    pass
import numpy as _np
cm.PROW = 102
cm.ITILE_ROWS = 55
def _pad_half(x_bchw, b, r0):
    C = x_bchw.shape[1]
    out = _np.zeros((C, 55, 102), _np.float32)
    lo, hi = r0 - 2, r0 + 53
    src_lo, src_hi = max(lo, 0), min(hi, 100)
    out[:, src_lo - lo:src_hi - lo, 1:101] = x_bchw[b, :, src_lo:src_hi, :]
    return out
def _rotpm_table(rot):
    cols = []
    for h in range(4):
        cols.append(rot[:, h, :])
        cols.append(-rot[:, h, :])
    return _np.ascontiguousarray(_np.concatenate(cols, axis=1).astype(_np.float32))
cm.pad_half = staticmethod(_pad_half)
cm.rotpm_table = staticmethod(_rotpm_table)


def make_l1_inputs(inputs, rot):
    """Build the 8 per-core input dicts from the problem inputs."""
    inp = {k: np.asarray(v) for k, v in inputs.items()}
    rotpm = cm.rotpm_table(rot)

    def wpack(w):
        Cout, Cin = w.shape[:2]
        p = np.stack([np.concatenate([w[:, :, 0, kx].T, w[:, :, 1, kx].T], axis=0)
                      for kx in range(3)]).astype(np.float32)
        s = np.stack([np.ascontiguousarray(w[:, :, 2, kx].T)
                      for kx in range(3)]).astype(np.float32)
        return p, s

    a1p, a1s = wpack(inp['a1w1'])
    a1q, a1t = wpack(inp['a1w2'])
    a2p, a2s = wpack(inp['a2w1'])
    a2q, a2t = wpack(inp['a2w2'])
    m1p, m1s = wpack(inp['mw1'])
    def wpack_gap(w):
        p = []
        for kx in range(3):
            m = np.zeros((64, 16), np.float32)
            m[0:16] = w[:, :, 0, kx].T
            m[32:48] = w[:, :, 1, kx].T
            p.append(m)
        s = np.stack([np.ascontiguousarray(w[:, :, 2, kx].T)
                      for kx in range(3)]).astype(np.float32)
        return np.stack(p), s
    m2p, m2s = wpack_gap(inp['mw2'])
    ida = np.eye(64, dtype=np.float32)
    wskip = np.ascontiguousarray(inp['mws'][:, :, 0, 0].T).astype(np.float32)

    # a-units: (src, w1, w2) ; m-units: src per (b, f1/f2)
    a_units = [(inp['feature_dec1'], 0, a1p, a1s, a1q, a1t),
               (inp['reference_feature'], 0, a2p, a2s, a2q, a2t),
               (inp['feature_dec1'], 1, a1p, a1s, a1q, a1t),
               (inp['reference_feature'], 1, a2p, a2s, a2q, a2t)]
    m_units = [(inp['feature_dec1'], 0), (inp['feature_dec2'], 0),
               (inp['feature_dec1'], 1), (inp['feature_dec2'], 1)]

    in_maps = []
    for c in range(8):
        au, ah = c // 2, c % 2
        asrc, ab, p1, s1, p2, s2 = a_units[au]
        msrc, mb = m_units[c // 2]
        d = {
            'xA': cm.pad_half(asrc, ab, ah * 50).reshape(64, -1),
            'xM': cm.pad_half(msrc, mb, (c % 2) * 50).reshape(64, -1),
            'wa1p': p1, 'wa1s': s1, 'wa2p': p2, 'wa2s': s2, 'ida': ida,
            'wm1p': m1p, 'wm1s': m1s, 'wm2p': m2p, 'wm2s': m2s,
            'wskip': wskip, 'rotpm': rotpm,
            'hmask': np.broadcast_to(np.array(
                [1.0 if (c % 2) == 1 else 0.0,
                 1.0 if (c % 2) == 0 else 0.0], np.float32), (128, 2)).copy(),
        }
        in_maps.append(d)
    return in_maps


N_HASHES, CHUNK, L, HB = 4, 144, 10000, 128
_CACHE = {}


def _run_cached(nc, in_maps, n_cores=8):
    """Like bass2jax.run_bass_via_pjrt but the jitted executable is memoized,
    so warm calls skip re-trace/compile of the PJRT program."""
    import jax
    import numpy as _np
    from jax.sharding import Mesh, PartitionSpec
    from jax.experimental.shard_map import shard_map
    from concourse import bass2jax, mybir as _mb

    if "exec" not in _CACHE:
        bass2jax.install_neuronx_cc_hook()
        in_names, out_names, out_avals, zero_shapes = [], [], [], []
        for alloc in nc.m.functions[0].allocations:
            if not isinstance(alloc, _mb.MemoryLocationSet):
                continue
            name = alloc.memorylocations[0].name
            if alloc.kind == "ExternalInput":
                in_names.append(name)
            elif alloc.kind == "ExternalOutput":
                out_names.append(name)
                shape = tuple(alloc.tensor_shape)
                dtype = _mb.dt.np(alloc.dtype)
                out_avals.append(jax.core.ShapedArray(shape, dtype))
                zero_shapes.append((shape, dtype))
        n_params = len(in_names)
        all_names = tuple(in_names + out_names)

        def _body(*args):
            outs = bass2jax._bass_exec_p.bind(
                *args, out_avals=tuple(out_avals), in_names=all_names,
                out_names=tuple(out_names), lowering_input_output_aliases=(),
                sim_require_finite=True, sim_require_nnan=True, nc=nc)
            return tuple(outs)

        devices = jax.devices()[:n_cores]
        mesh = Mesh(_np.asarray(devices), ("core",))
        n_out = len(out_names)
        sharded = jax.jit(
            shard_map(_body, mesh=mesh,
                      in_specs=(PartitionSpec("core"),) * (n_params + n_out),
                      out_specs=(PartitionSpec("core"),) * n_out,
                      check_rep=False),
            donate_argnums=tuple(range(n_params, n_params + n_out)),
            keep_unused=True)
        _CACHE["exec"] = (sharded, in_names, out_names, out_avals, zero_shapes)

    sharded, in_names, out_names, out_avals, zero_shapes = _CACHE["exec"]
    concat_in = [_np.concatenate([_np.asarray(m[name]) for m in in_maps], axis=0)
                 for name in in_names]
    concat_zeros = [_np.zeros((n_cores * s[0], *s[1:]), d) for s, d in zero_shapes]
    out_arrs = sharded(*concat_in, *concat_zeros)
    return [
        {name: _np.asarray(out_arrs[i]).reshape(n_cores, *out_avals[i].shape)[c]
         for i, name in enumerate(out_names)}
        for c in range(n_cores)
    ]



def _get_nc():
    if "nc" not in _CACHE:
        _CACHE["nc"] = build_l1()
    return _CACHE["nc"]


def kernel(**inputs):
    import jax, time
    from concourse.bass_utils import run_bass_kernel_spmd
    inp = {k: np.asarray(v) for k, v in inputs.items()}
    ri = inp["random_index"].astype(np.int64)
    rot = np.asarray(jax.random.normal(jax.random.key(42), (16, N_HASHES, HB // 2),
                                       dtype=jax.numpy.float32))
    nc = _get_nc()
    in_maps = make_l1_inputs(inp, rot)
    _t0 = time.time()
    try:
        res = _run_cached(nc, in_maps)
    except Exception:
        _CACHE.pop("exec", None)
        res = run_bass_kernel_spmd(nc, in_maps, list(range(8))).results
    _CACHE["device_wall_ns"] = int((time.time() - _t0) * 1e9)

    codes = np.zeros((2, N_HASHES, 2 * L), np.int64)
    fcm = np.zeros((2, 16, 2 * L), np.float32)
    acm = np.zeros((2, 64, 2 * L), np.float32)
    for c in range(8):
        b, q = c // 4, c % 4
        cu = res[c]["codesU"]
        for blk in range(40):
            m = min(128, 5000 - blk * 128)
            for h in range(N_HASHES):
                codes[b, h, q * 5000 + blk * 128:q * 5000 + blk * 128 + m] = \
                    cu[0:m, (blk * 4 + h) * 8]
        fcm[b, :, q * 5000:(q + 1) * 5000] = res[c]["fY"]
        au, ah = c // 2, c % 2
        apart = 0 if au < 2 else 1          # batch of this a-unit
        off = (au % 2) * L                  # a1 -> aux1 half, a2 -> refe half
        acm[apart, :, off + ah * 5000:off + (ah + 1) * 5000] = res[c]["aY"]

    out = np.zeros((2, 64, L), np.float32)
    tt = np.arange(2 * L)
    X = (tt % 2).astype(np.int64)
    jA, jB = ri[0], ri[L]
    padk = CHUNK - (2 * L) % CHUNK
    kch = (2 * L + padk) // CHUNK
    for b in range(2):
        qA, qB = fcm[b, :, jA], fcm[b, :, jB]
        zA = 0.01 if jA < L else 0.99
        zB = 0.01 if jB < L else 0.99
        rA, rB = acm[b, :, jA], acm[b, :, jB]
        nh = lambda v: v / max(np.sqrt(np.sum(v.astype(np.float64) ** 2)), 5e-5)
        Ah, Bh = nh(qA), nh(qB)
        s = np.array([[qA @ Ah, qA @ Bh], [qB @ Ah, qB @ Bh]])
        Asum = np.zeros(2 * L, np.float64)
        for h in range(N_HASHES):
            cp = codes[b, h][ri]
            order = np.argsort(cp, kind="stable")
            slot = np.empty(2 * L, np.int64)
            slot[order] = tt
            chunk = slot // CHUNK
            ev = X == 0
            na = np.bincount(chunk[ev], minlength=kch).astype(np.int64)
            na[kch - 1] += np.count_nonzero((slot >= 2 * L - padk) & ev)
            na3 = na + np.roll(na, 1) + np.roll(na, -1)
            Asum += na3[chunk]
        eA = np.exp(s[:, 0])[X] * zA
        eB = np.exp(s[:, 1])[X] * zB
        u = Asum * eA
        v = (N_HASHES * 3 * CHUNK - Asum) * eB
        w = (u / (u + v)).astype(np.float32)
        comb = w[:, None] * rA[None, :] + (1 - w)[:, None] * rB[None, :]
        keep = ri < L
        out[b][:, ri[keep]] = comb[keep].T
    return out.reshape(2, 64, 100, 100)



# revision 2
# speedup vs baseline: 5.6212x; 5.6212x over previous
"""Trainium2 kernel for nn_Non_Local_Sparse_Network (sparse_attention).

The attention algebra collapses to rank-2: mod_indices = (indices % 2) * L
means every query/value row of the bucketed attention is one of just two
vectors (rows ri[0] and ri[L] of the permuted embeddings).  So the final
output is w*rA + (1-w)*rB per position, where only the LSH bucket codes
(from the 16-channel m-resblock embeddings) require full-image compute.

Device (8 NeuronCores, SPMD, one (batch, f1/f2, half) m-unit per core):
int16-dequant + the m-resblock convs in exact f32 (TensorE), the LSH
rv = f @ [rot|-rot] matmul, per-hash argmax codes (DVE max/max_index),
plus the top1-top2 gap so the host can exactly re-resolve near-ties.
Host: 5x5-patch resblocks for the four distinguished pixels (qA/qB/rA/rB),
gap-thresholded exact fixup of bucket codes, and the O(N) counting-sort
combine.  Only ~6.5MB up / ~0.5MB down crosses the device link.
"""
import numpy as np

"""Patch TileContext._drain_and_barrier: this walrus build only accepts one
sync-wait on an SP Drain, so split the tail drain's waits across a chain of
single-wait drains."""
import bass_rust
import concourse.tile as _tile
from concourse.vector_clock import ScopedClock


def _drain_and_barrier_split(self, tick_clock, wait_clock):
    drain_inst = self.nc.sync.drain()
    wait_clock.add_sem_waits(
        drain_inst.ins, ScopedClock({None: tick_clock.global_clock})
    )
    si = drain_inst.ins.sync_info
    waits = list(si.on_wait)
    if len(waits) > 1:
        drain_inst.ins.sync_info = bass_rust.SyncInfo(
            on_wait=[waits[0]], on_update=list(si.on_update)
        )
        for w in waits[1:]:
            extra = self.nc.sync.drain()
            extra.ins.sync_info = bass_rust.SyncInfo(on_wait=[w], on_update=[])

    self.nc.all_engine_barrier()
    assert self.sems is not None
    popped = self.nc._tile_sem_poison_stack.pop()
    assert popped is self._sem_poison
    self.nc.clear_and_free_semaphores(list(self.sems.allocated().values()))
    self.nc.all_engine_barrier()


_tile.TileContext._drain_and_barrier = _drain_and_barrier_split


def legalize_single_wait(nc):
    """This walrus build allows at most one sync-wait per instruction.
    For any instruction carrying k>1 waits, hoist k-1 of them onto fresh
    same-engine NOPs inserted immediately before it (same-engine program
    order makes this semantically identical)."""
    import concourse.mybir as mybir

    def make_nop(engine_type):
        eng = nc.engines[engine_type]
        binst = eng.nop()
        ins = binst.ins
        # eng.nop() appended to the current bb; pull it back out
        for fn in nc.m.functions:
            for bb in fn.blocks:
                il = bb.instructions
                if il and il[-1] is ins:
                    del il[-1]
                    return ins
        raise RuntimeError("fresh nop not found at tail of any bb")

    n_fixed = 0
    for fn in nc.m.functions:
        for bb in fn.blocks:
            il = bb.instructions
            i = 0
            while i < len(il):
                inst = il[i]
                try:
                    si = inst.sync_info
                except Exception:
                    si = None
                if si is None:
                    i += 1
                    continue
                waits = list(si.on_wait)
                if len(waits) > 1:
                    for w in waits[:-1]:
                        nop = make_nop(inst.engine)
                        nop.sync_info = bass_rust.SyncInfo(on_wait=[w], on_update=[])
                        il.insert(i, nop)
                        i += 1
                    inst.sync_info = bass_rust.SyncInfo(
                        on_wait=[waits[-1]], on_update=list(si.on_update)
                    )
                    n_fixed += 1
                i += 1
    return n_fixed


import concourse.bass as bass
import concourse.mybir as mybir
import concourse.tile as tile
from contextlib import ExitStack
from numpy.lib.stride_tricks import sliding_window_view

F32 = mybir.dt.float32
F16 = mybir.dt.float16
I16 = mybir.dt.int16
U16 = mybir.dt.uint16
U8 = mybir.dt.uint8
PROW = 102
NROWS = 55
NFLAT = NROWS * PROW  # 5610
AF = mybir.ActivationFunctionType

N_HASHES, CHUNK, L, HB = 4, 144, 10000, 128
SCALE = 6.0 / 32767.0
TAU = 8e-3  # gap threshold for host-side exact re-resolution of codes
_CACHE = {}


def build_l1():
    nc = bass.Bass("TRN2", target_bir_lowering=False, debug=False, num_devices=8)
    xM = nc.dram_tensor("xM", [64, NFLAT], I16, kind="ExternalInput")
    wm1p = nc.dram_tensor("wm1p", [3, 128, 16], F32, kind="ExternalInput")
    wm1s = nc.dram_tensor("wm1s", [3, 64, 16], F32, kind="ExternalInput")
    wm2p = nc.dram_tensor("wm2p", [3, 64, 16], F32, kind="ExternalInput")
    wm2s = nc.dram_tensor("wm2s", [3, 16, 16], F32, kind="ExternalInput")
    wskip = nc.dram_tensor("wskip", [64, 16], F32, kind="ExternalInput")
    rotpm = nc.dram_tensor("rotpm", [16, 512], F32, kind="ExternalInput")
    hmask = nc.dram_tensor("hmask", [128, 2], F32, kind="ExternalInput")

    codesO = nc.dram_tensor("codesO", [128, 160], U8, kind="ExternalOutput")
    gapO = nc.dram_tensor("gapO", [128, 160], F16, kind="ExternalOutput")

    with tile.TileContext(nc) as tc, ExitStack() as ctx:
        const = ctx.enter_context(tc.tile_pool(name="const", bufs=1))
        big = ctx.enter_context(tc.tile_pool(name="big", bufs=1))
        work = ctx.enter_context(tc.tile_pool(name="work", bufs=3))
        psum = ctx.enter_context(tc.tile_pool(name="psum", bufs=2, space="PSUM"))
        pidx = ctx.enter_context(tc.tile_pool(name="pidx", bufs=2, space="PSUM"))

        # dequant int16 -> f32 into the row-paired layout (rows 64:128 are
        # the image shifted down one row, so conv ky=0,1 fuse into one matmul)
        ti = big.tile([64, NFLAT], I16)
        nc.sync.dma_start(out=ti[:], in_=xM[:])
        xmR = big.tile([128, NFLAT], F32)
        nc.scalar.mul(xmR[0:64, :], ti[:], SCALE)
        nc.scalar.mul(xmR[64:128, 0:NFLAT - PROW], ti[:, PROW:NFLAT], SCALE)

        def load_w(src, k, cout, tag):
            t = const.tile([k, cout], F32, tag=tag)
            nc.sync.dma_start(out=t[:], in_=src)
            return t

        wm1 = [load_w(wm1p[i], 128, 16, f"wm1p{i}") for i in range(3)] + \
              [load_w(wm1s[i], 64, 16, f"wm1s{i}") for i in range(3)]
        wm2 = [load_w(wm2p[i], 64, 16, f"wm2p{i}") for i in range(3)] + \
              [load_w(wm2s[i], 16, 16, f"wm2s{i}") for i in range(3)]
        wskipR = load_w(wskip[:, :], 64, 16, "wskip")
        rot_t = load_w(rotpm[:, :], 16, 512, "rot")
        hm = load_w(hmask[:, :], 128, 2, "hm")

        def mask_h1(h1r, cout):
            # zero conv1 rows at image row -1 (u=0, half0) / 100 (u=51, half1)
            AL = mybir.AluOpType
            nc.vector.tensor_scalar(out=h1r[0:cout, 0:PROW],
                                    in0=h1r[0:cout, 0:PROW],
                                    scalar1=hm[0:cout, 0:1], scalar2=None, op0=AL.mult)
            nc.vector.tensor_scalar(out=h1r[0:cout, 51 * PROW:52 * PROW],
                                    in0=h1r[0:cout, 51 * PROW:52 * PROW],
                                    scalar1=hm[0:cout, 1:2], scalar2=None, op0=AL.mult)
            p2 = 64 if cout == 64 else 32
            nc.vector.tensor_scalar(out=h1r[p2:p2 + cout, 50 * PROW:51 * PROW],
                                    in0=h1r[p2:p2 + cout, 50 * PROW:51 * PROW],
                                    scalar1=hm[0:cout, 1:2], scalar2=None, op0=AL.mult)

        def r3(ap, nrowstile):
            return ap.rearrange("p (r c) -> p r c", c=PROW)

        # (drow, dcol, K) per matmul: 3 paired (ky=0&1) + 3 single (ky=2)
        def offs(cin):
            return [(0, kx, 2 * cin) for kx in range(3)] + \
                   [(2, kx, cin) for kx in range(3)]

        def conv1(xr, wts, om, cout, h1r, poff):
            x3 = r3(xr, NROWS)
            for j in range(11):
                y0 = 5 * j
                nrow = min(5, 52 - y0)
                n = nrow * 100
                pfull = psum.tile([64, 500], F32, tag="pconv")
                p = pfull[0:cout, :]
                for i, (dr, dc, k) in enumerate(om):
                    rhs = x3[0:k, y0 + dr:y0 + dr + nrow, dc:dc + 100]
                    nc.tensor.matmul(p[:, 0:n], wts[i][:], rhs,
                                     start=(i == 0), stop=(i == len(om) - 1))
                ps = p[:, 0:n].rearrange("p (r c) -> p r c", c=100)
                h3a = r3(h1r[0:cout, :], 52)
                nc.scalar.activation(h3a[:, y0:y0 + nrow, 1:101], ps, AF.Relu)
                h3b = r3(h1r[poff:poff + cout, :], 52)
                if j == 0:
                    ps1 = p[:, 100:n].rearrange("p (r c) -> p r c", c=100)
                    nc.scalar.activation(h3b[:, 0:nrow - 1, 1:101], ps1, AF.Relu)
                else:
                    nc.scalar.activation(h3b[:, y0 - 1:y0 - 1 + nrow, 1:101], ps, AF.Relu)

        def conv2(h1r, wtom, cout, elh, ext, k2, outdst):
            wts, om = wtom
            h3 = r3(h1r, 52)
            e3 = r3(ext, NROWS)
            for j in range(10):
                z0 = 5 * j
                pfull = psum.tile([64, 500], F32, tag="pconv")
                p = pfull[0:cout, :]
                for i, (dr, dc, k) in enumerate(om):
                    rhs = h3[0:k, z0 + dr:z0 + dr + 5, dc:dc + 100]
                    nc.tensor.matmul(p[:], wts[i][:], rhs, start=(i == 0), stop=False)
                rhs = e3[0:k2, z0 + 2:z0 + 7, 1:101]
                nc.tensor.matmul(p[:], elh[:], rhs, start=False, stop=True)
                nc.scalar.copy(outdst[:, 500 * j:500 * j + 500], p[:])

        h1mR = big.tile([64, 52 * PROW], F32)
        nc.vector.memset(h1mR[:], 0.0)
        fYt = big.tile([16, 5000], F32)

        m2om = (wm2, [(0, kx, 64) for kx in range(3)] + [(2, kx, 16) for kx in range(3)])
        conv1(xmR, wm1, offs(64), 16, h1mR, 32)
        mask_h1(h1mR, 16)
        conv2(h1mR, m2om, 16, wskipR, xmR, 64, fYt)

        codesT = big.tile([128, 1280], U16)
        nc.vector.memset(codesT[:], 0)
        gapT = big.tile([128, 160], F16)
        nc.vector.memset(gapT[:], 0)
        for blk in range(40):
            m = min(128, 5000 - blk * 128)
            pr = pidx.tile([128, 512], F32, tag="rv")
            nc.tensor.matmul(pr[0:m, :], fYt[:, blk * 128:blk * 128 + m],
                             rot_t[:], start=True, stop=True)
            rvsb = work.tile([128, 512], F32, tag="rvsb")
            nc.vector.tensor_copy(rvsb[0:m, :], pr[0:m, :])
            for h in range(4):
                col = blk * 4 + h
                mx = work.tile([128, 8], F32, tag="mx")
                nc.vector.max(mx[0:m, :], rvsb[0:m, h * 128:(h + 1) * 128])
                nc.vector.max_index(codesT[0:m, col * 8:col * 8 + 8],
                                    mx[0:m, :], rvsb[0:m, h * 128:(h + 1) * 128])
                nc.vector.tensor_sub(gapT[0:m, col:col + 1],
                                     mx[0:m, 0:1], mx[0:m, 1:2])
        codes8 = big.tile([128, 160], U8)
        nc.vector.tensor_copy(codes8[:].rearrange("p (c e) -> p c e", e=1),
                              codesT[:].rearrange("p (c e) -> p c e", e=8)[:, :, 0:1])
        nc.sync.dma_start(out=codesO[:], in_=codes8[:])
        nc.sync.dma_start(out=gapO[:], in_=gapT[:])

    legalize_single_wait(nc)
    return nc


# ---- host-side input prep ----
def _pad_half_i16(q_bchw, b, r0):
    C = q_bchw.shape[1]
    out = np.zeros((C, NROWS, PROW), np.int16)
    lo, hi = r0 - 2, r0 + 53
    src_lo, src_hi = max(lo, 0), min(hi, 100)
    out[:, src_lo - lo:src_hi - lo, 1:101] = q_bchw[b, :, src_lo:src_hi, :]
    return out


def _rotpm_table(rot):
    cols = []
    for h in range(N_HASHES):
        cols.append(rot[:, h, :])
        cols.append(-rot[:, h, :])
    return np.ascontiguousarray(np.concatenate(cols, axis=1).astype(np.float32))


def make_l1_inputs(inp, rot):
    """Build the 8 per-core input dicts from the problem inputs."""
    rotpm = _rotpm_table(rot)

    def wpack(w):
        p = np.stack([np.concatenate([w[:, :, 0, kx].T, w[:, :, 1, kx].T], axis=0)
                      for kx in range(3)]).astype(np.float32)
        s = np.stack([np.ascontiguousarray(w[:, :, 2, kx].T)
                      for kx in range(3)]).astype(np.float32)
        return p, s

    def wpack_gap(w):
        p = []
        for kx in range(3):
            m = np.zeros((64, 16), np.float32)
            m[0:16] = w[:, :, 0, kx].T
            m[32:48] = w[:, :, 1, kx].T
            p.append(m)
        s = np.stack([np.ascontiguousarray(w[:, :, 2, kx].T)
                      for kx in range(3)]).astype(np.float32)
        return np.stack(p), s

    m1p, m1s = wpack(inp['mw1'])
    m2p, m2s = wpack_gap(inp['mw2'])
    wskip = np.ascontiguousarray(inp['mws'][:, :, 0, 0].T).astype(np.float32)

    q1 = np.rint(inp['feature_dec1'] * (1.0 / SCALE)).astype(np.int16)
    q2 = np.rint(inp['feature_dec2'] * (1.0 / SCALE)).astype(np.int16)
    m_units = [(q1, 0), (q2, 0), (q1, 1), (q2, 1)]

    in_maps = []
    for c in range(8):
        msrc, mb = m_units[c // 2]
        half = c % 2
        d = {
            'xM': _pad_half_i16(msrc, mb, half * 50).reshape(64, -1),
            'wm1p': m1p, 'wm1s': m1s, 'wm2p': m2p, 'wm2s': m2s,
            'wskip': wskip, 'rotpm': rotpm,
            'hmask': np.broadcast_to(np.array(
                [1.0 if half == 1 else 0.0,
                 1.0 if half == 0 else 0.0], np.float32), (128, 2)).copy(),
        }
        in_maps.append(d)
    return in_maps


def _run_cached(nc, in_maps, n_cores=8):
    """run_bass_via_pjrt with the jitted executable memoized, so warm calls
    skip re-trace/lowering of the PJRT program."""
    import jax
    from jax.sharding import Mesh, PartitionSpec
    from jax.experimental.shard_map import shard_map
    from concourse import bass2jax, mybir as _mb

    if "exec" not in _CACHE:
        bass2jax.install_neuronx_cc_hook()
        part_name = nc.partition_id_tensor.name if nc.partition_id_tensor else None
        in_names, out_names, out_avals, zero_shapes = [], [], [], []
        for alloc in nc.m.functions[0].allocations:
            if not isinstance(alloc, _mb.MemoryLocationSet):
                continue
            name = alloc.memorylocations[0].name
            if alloc.kind == "ExternalInput":
                if name != part_name:
                    in_names.append(name)
            elif alloc.kind == "ExternalOutput":
                out_names.append(name)
                shape = tuple(alloc.tensor_shape)
                dtype = _mb.dt.np(alloc.dtype)
                out_avals.append(jax.core.ShapedArray(shape, dtype))
                zero_shapes.append((shape, dtype))
        n_params = len(in_names)
        bind_names = tuple(in_names + out_names + ([part_name] if part_name else []))

        def _body(*args):
            operands = list(args)
            if part_name:
                operands.append(bass2jax.partition_id_tensor())
            outs = bass2jax._bass_exec_p.bind(
                *operands, out_avals=tuple(out_avals), in_names=bind_names,
                out_names=tuple(out_names), lowering_input_output_aliases=(),
                sim_require_finite=True, sim_require_nnan=True, nc=nc)
            return tuple(outs)

        devices = jax.devices()[:n_cores]
        mesh = Mesh(np.asarray(devices), ("core",))
        n_out = len(out_names)
        sharded = jax.jit(
            shard_map(_body, mesh=mesh,
                      in_specs=(PartitionSpec("core"),) * (n_params + n_out),
                      out_specs=(PartitionSpec("core"),) * n_out,
                      check_rep=False),
            donate_argnums=tuple(range(n_params, n_params + n_out)),
            keep_unused=True)
        _CACHE["exec"] = (sharded, in_names, out_names, out_avals, zero_shapes)

    sharded, in_names, out_names, out_avals, zero_shapes = _CACHE["exec"]
    concat_in = [np.concatenate([np.asarray(m[name]) for m in in_maps], axis=0)
                 for name in in_names]
    concat_zeros = [np.zeros((n_cores * s[0], *s[1:]), d) for s, d in zero_shapes]
    out_arrs = sharded(*concat_in, *concat_zeros)
    return [
        {name: np.asarray(out_arrs[i]).reshape(n_cores, *out_avals[i].shape)[c]
         for i, name in enumerate(out_names)}
        for c in range(n_cores)
    ]


def _get_nc():
    if "nc" not in _CACHE:
        _CACHE["nc"] = build_l1()
    return _CACHE["nc"]


# ---- host-side exact pixel resblock (vectorized over positions) ----
def respix_batch(x_chw, pts, w1, b1, w2, b2, ws=None, bs=None):
    """Exact resblock outputs at flat pixel indices pts of one image.
    x_chw: (Ci,100,100) f32; returns (P, Co) f32."""
    pts = np.asarray(pts, np.int64)
    P = len(pts)
    y, xc = pts // 100, pts % 100
    Ci = x_chw.shape[0]
    xp = np.zeros((Ci, 104, 104), np.float32)
    xp[:, 2:102, 2:102] = x_chw
    rows = y[:, None] + np.arange(5)[None, :]
    cols = xc[:, None] + np.arange(5)[None, :]
    patches = xp[:, rows[:, :, None], cols[:, None, :]]       # (Ci,P,5,5)
    win = sliding_window_view(patches, (3, 3), axis=(2, 3))   # (Ci,P,3,3,3,3)
    xm = win.transpose(1, 2, 3, 0, 4, 5).reshape(P * 9, Ci * 9)
    W1 = w1.reshape(w1.shape[0], Ci * 9).astype(np.float32)
    h = (xm @ W1.T).reshape(P, 3, 3, -1) + b1.astype(np.float32)
    uval = ((y[:, None] - 1 + np.arange(3)[None, :]) >= 0) & \
           ((y[:, None] - 1 + np.arange(3)[None, :]) < 100)
    vval = ((xc[:, None] - 1 + np.arange(3)[None, :]) >= 0) & \
           ((xc[:, None] - 1 + np.arange(3)[None, :]) < 100)
    h = h * uval[:, :, None, None] * vval[:, None, :, None]
    h = np.maximum(h, 0)
    W2 = w2.transpose(0, 2, 3, 1).reshape(w2.shape[0], -1).astype(np.float32)
    out = h.reshape(P, -1) @ W2.T + b2.astype(np.float32)
    x_c = x_chw[:, y, xc].T                                   # (P,Ci)
    if ws is None:
        out = out + x_c
    else:
        out = out + x_c @ ws[:, :, 0, 0].T.astype(np.float32) + bs.astype(np.float32)
    return out


def _q_at(inp, b, t):
    """Exact m-resblock embedding (16,) at f1f2 position t of batch b."""
    src = inp['feature_dec1'] if t < L else inp['feature_dec2']
    return respix_batch(src[b], [t % L], inp['mw1'], inp['mb1'],
                        inp['mw2'], inp['mb2'], inp['mws'], inp['mbs'])[0]


def _r_at(inp, b, t):
    """Exact [aux1|refe] embedding (64,) at concat position t of batch b."""
    if t < L:
        return respix_batch(inp['feature_dec1'][b], [t], inp['a1w1'], inp['a1b1'],
                            inp['a1w2'], inp['a1b2'])[0]
    return respix_batch(inp['reference_feature'][b], [t - L], inp['a2w1'],
                        inp['a2b1'], inp['a2w2'], inp['a2b2'])[0]


def kernel(**inputs):
    import jax, time
    inp = {k: np.asarray(v) for k, v in inputs.items()}
    ri = inp["random_index"].astype(np.int64)
    rot = np.asarray(jax.random.normal(jax.random.key(42), (16, N_HASHES, HB // 2),
                                       dtype=jax.numpy.float32))
    nc = _get_nc()
    in_maps = make_l1_inputs(inp, rot)
    _t0 = time.time()
    try:
        res = _run_cached(nc, in_maps)
    except Exception:
        _CACHE.pop("exec", None)
        from concourse.bass_utils import run_bass_kernel_spmd
        res = run_bass_kernel_spmd(nc, in_maps, list(range(8))).results
    _CACHE["device_wall_ns"] = int((time.time() - _t0) * 1e9)

    codes = np.zeros((2, N_HASHES, 2 * L), np.int64)
    gaps = np.ones((2, N_HASHES, 2 * L), np.float32)
    for c in range(8):
        b, q = c // 4, c % 4
        cu = res[c]["codesO"]
        gu = res[c]["gapO"].astype(np.float32)
        for h in range(N_HASHES):
            codes[b, h, q * 5000:(q + 1) * 5000] = \
                cu[:, h::4].astype(np.int64).T.reshape(-1)[:5000]
            gaps[b, h, q * 5000:(q + 1) * 5000] = gu[:, h::4].T.reshape(-1)[:5000]

    # exact fixup of near-tie argmaxes (quantization could have flipped them)
    for b in range(2):
        sus = gaps[b] < TAU                       # (nh, 2L)
        pos = np.where(sus.any(axis=0))[0]
        if len(pos) == 0:
            continue
        f_ex = np.zeros((len(pos), 16), np.float32)
        lo = pos < L
        if lo.any():
            f_ex[lo] = respix_batch(inp['feature_dec1'][b], pos[lo],
                                    inp['mw1'], inp['mb1'], inp['mw2'], inp['mb2'],
                                    inp['mws'], inp['mbs'])
        if (~lo).any():
            f_ex[~lo] = respix_batch(inp['feature_dec2'][b], pos[~lo] - L,
                                     inp['mw1'], inp['mb1'], inp['mw2'], inp['mb2'],
                                     inp['mws'], inp['mbs'])
        for h in range(N_HASHES):
            hs = np.where(sus[h])[0]
            if len(hs) == 0:
                continue
            fv = f_ex[np.searchsorted(pos, hs)]
            rv = fv @ rot[:, h, :]
            codes[b, h, hs] = np.argmax(np.concatenate([rv, -rv], axis=1), axis=1)

    # rank-2 combine: out = w*rA + (1-w)*rB per position
    out = np.zeros((2, 64, L), np.float32)
    tt = np.arange(2 * L)
    X = (tt % 2).astype(np.int64)
    jA, jB = int(ri[0]), int(ri[L])
    padk = CHUNK - (2 * L) % CHUNK
    kch = (2 * L + padk) // CHUNK
    for b in range(2):
        qA, qB = _q_at(inp, b, jA).astype(np.float64), _q_at(inp, b, jB).astype(np.float64)
        zA = 0.01 if jA < L else 0.99
        zB = 0.01 if jB < L else 0.99
        rA, rB = _r_at(inp, b, jA), _r_at(inp, b, jB)
        nh = lambda v: v / max(np.sqrt(np.sum(v ** 2)), 5e-5)
        Ah, Bh = nh(qA), nh(qB)
        s = np.array([[qA @ Ah, qA @ Bh], [qB @ Ah, qB @ Bh]])
        Asum = np.zeros(2 * L, np.float64)
        for h in range(N_HASHES):
            cp = codes[b, h][ri]
            order = np.argsort(cp, kind="stable")
            slot = np.empty(2 * L, np.int64)
            slot[order] = tt
            chunk = slot // CHUNK
            ev = X == 0
            na = np.bincount(chunk[ev], minlength=kch).astype(np.int64)
            na[kch - 1] += np.count_nonzero((slot >= 2 * L - padk) & ev)
            na3 = na + np.roll(na, 1) + np.roll(na, -1)
            Asum += na3[chunk]
        eA = np.exp(s[:, 0])[X] * zA
        eB = np.exp(s[:, 1])[X] * zB
        u = Asum * eA
        v = (N_HASHES * 3 * CHUNK - Asum) * eB
        w = (u / (u + v)).astype(np.float32)
        comb = w[:, None] * rA[None, :].astype(np.float32) + \
               (1 - w)[:, None] * rB[None, :].astype(np.float32)
        keep = ri < L
        out[b][:, ri[keep]] = comb[keep].T
    return out.reshape(2, 64, 100, 100)


# revision 10
# speedup vs baseline: 7.4908x; 1.3326x over previous
"""Trainium2 kernel for nn_Non_Local_Sparse_Network (sparse_attention).

The attention algebra collapses to rank-2: mod_indices = (indices % 2) * L
means every query/value row of the bucketed attention is one of just two
vectors (rows ri[0] and ri[L] of the permuted embeddings).  So the final
output is w*rA + (1-w)*rB per position, where only the LSH bucket codes
(from the 16-channel m-resblock embeddings) require full-image compute.

Device (8 NeuronCores, SPMD, one (batch, f1/f2, half) m-unit per core):
int16-dequant + the m-resblock convs in exact f32 (TensorE), the LSH
rv = f @ [rot|-rot] matmul, per-hash argmax codes (DVE max/max_index),
plus the top1-top2 gap so the host can exactly re-resolve near-ties.
Host: 5x5-patch resblocks for the four distinguished pixels (qA/qB/rA/rB),
gap-thresholded exact fixup of bucket codes, and the O(N) counting-sort
combine.  Only ~6.5MB up / ~0.5MB down crosses the device link.
"""
import numpy as np

"""Patch TileContext._drain_and_barrier: this walrus build only accepts one
sync-wait on an SP Drain, so split the tail drain's waits across a chain of
single-wait drains."""
import bass_rust
import concourse.tile as _tile
from concourse.vector_clock import ScopedClock


def _drain_and_barrier_split(self, tick_clock, wait_clock):
    drain_inst = self.nc.sync.drain()
    wait_clock.add_sem_waits(
        drain_inst.ins, ScopedClock({None: tick_clock.global_clock})
    )
    si = drain_inst.ins.sync_info
    waits = list(si.on_wait)
    if len(waits) > 1:
        drain_inst.ins.sync_info = bass_rust.SyncInfo(
            on_wait=[waits[0]], on_update=list(si.on_update)
        )
        for w in waits[1:]:
            extra = self.nc.sync.drain()
            extra.ins.sync_info = bass_rust.SyncInfo(on_wait=[w], on_update=[])

    self.nc.all_engine_barrier()
    assert self.sems is not None
    popped = self.nc._tile_sem_poison_stack.pop()
    assert popped is self._sem_poison
    self.nc.clear_and_free_semaphores(list(self.sems.allocated().values()))
    self.nc.all_engine_barrier()


_tile.TileContext._drain_and_barrier = _drain_and_barrier_split


def legalize_single_wait(nc):
    """This walrus build allows at most one sync-wait per instruction.
    For any instruction carrying k>1 waits, hoist k-1 of them onto fresh
    same-engine NOPs inserted immediately before it (same-engine program
    order makes this semantically identical)."""
    import concourse.mybir as mybir

    def make_nop(engine_type):
        eng = nc.engines[engine_type]
        binst = eng.nop()
        ins = binst.ins
        # eng.nop() appended to the current bb; pull it back out
        for fn in nc.m.functions:
            for bb in fn.blocks:
                il = bb.instructions
                if il and il[-1] is ins:
                    del il[-1]
                    return ins
        raise RuntimeError("fresh nop not found at tail of any bb")

    n_fixed = 0
    for fn in nc.m.functions:
        for bb in fn.blocks:
            il = bb.instructions
            i = 0
            while i < len(il):
                inst = il[i]
                try:
                    si = inst.sync_info
                except Exception:
                    si = None
                if si is None:
                    i += 1
                    continue
                waits = list(si.on_wait)
                if len(waits) > 1:
                    for w in waits[:-1]:
                        nop = make_nop(inst.engine)
                        nop.sync_info = bass_rust.SyncInfo(on_wait=[w], on_update=[])
                        il.insert(i, nop)
                        i += 1
                    inst.sync_info = bass_rust.SyncInfo(
                        on_wait=[waits[-1]], on_update=list(si.on_update)
                    )
                    n_fixed += 1
                i += 1
    return n_fixed


import concourse.bass as bass
import concourse.mybir as mybir
import concourse.tile as tile
from contextlib import ExitStack
from numpy.lib.stride_tricks import sliding_window_view

F32 = mybir.dt.float32
F16 = mybir.dt.float16
I16 = mybir.dt.int16
U16 = mybir.dt.uint16
U8 = mybir.dt.uint8
PROW = 102
NROWS = 55
NFLAT = NROWS * PROW  # 5610
AF = mybir.ActivationFunctionType

N_HASHES, CHUNK, L, HB = 4, 144, 10000, 128
SCALE = 6.0 / 32767.0
TAU = 8e-3  # gap threshold for host-side exact re-resolution of codes
_CACHE = {}

# single-input blob layout (int16 columns; f32 payloads bitcast to i16 pairs)
IMGC = 2805            # half of NFLAT: image ships as [128, 2805]
WB1 = 2806             # wm1p: 3 x [128,16] f32
WB2 = WB1 + 96         # wm1s: 3 x [64,16] f32
WB3 = WB2 + 96         # wm2p: 3 x [64,16] f32 (gap layout)
WB4 = WB3 + 96         # wm2s: 3 x [16,16] f32
WB5 = WB4 + 96         # wskip: [64,16] f32
WB6 = WB5 + 32         # hmask: [128,2] f32
WB7 = WB6 + 4          # rotpm: [16,512] f32 as [128,128] i16
NBLOB = WB7 + 128


def build_l1():
    nc = bass.Bass("TRN2", target_bir_lowering=False, debug=False, num_devices=8)
    blob = nc.dram_tensor("blob", [128, NBLOB], I16, kind="ExternalInput")
    outO = nc.dram_tensor("outO", [128, 480], U8, kind="ExternalOutput")

    with tile.TileContext(nc) as tc, ExitStack() as ctx:
        big = ctx.enter_context(tc.tile_pool(name="big", bufs=1))
        work = ctx.enter_context(tc.tile_pool(name="work", bufs=3))
        psum = ctx.enter_context(tc.tile_pool(name="psum", bufs=2, space="PSUM"))
        pidx = ctx.enter_context(tc.tile_pool(name="pidx", bufs=2, space="PSUM"))

        tb = big.tile([128, NBLOB], I16)
        nc.sync.dma_start(out=tb[:], in_=blob[:])
        # rot table re-read from DRAM into its [16,512] layout
        rt16 = big.tile([16, 1024], I16)
        nc.sync.dma_start(out=rt16[:].rearrange("p (a b) -> p a b", b=128),
                          in_=blob[:, WB7:WB7 + 128].rearrange("(q a) c -> q a c", a=8))
        rot_t = rt16[:].bitcast(F32)

        # dequant int16 -> f32 into the row-paired layout (rows 64:128 are
        # the image shifted down one row, so conv ky=0,1 fuse into one matmul)
        xmR = big.tile([128, NFLAT], F32)
        nc.scalar.mul(xmR[0:64, 0:IMGC], tb[0:64, 0:IMGC], SCALE)
        nc.scalar.mul(xmR[0:64, IMGC:2 * IMGC], tb[64:128, 0:IMGC], SCALE)
        nc.scalar.mul(xmR[64:128, 0:IMGC - PROW], tb[0:64, PROW:IMGC], SCALE)
        nc.scalar.mul(xmR[64:128, IMGC - PROW:2 * IMGC - PROW], tb[64:128, 0:IMGC], SCALE)

        wm1 = [tb[0:128, WB1 + 32 * i:WB1 + 32 * i + 32].bitcast(F32) for i in range(3)] + \
              [tb[0:64, WB2 + 32 * i:WB2 + 32 * i + 32].bitcast(F32) for i in range(3)]
        wm2 = [tb[0:64, WB3 + 32 * i:WB3 + 32 * i + 32].bitcast(F32) for i in range(3)] + \
              [tb[0:16, WB4 + 32 * i:WB4 + 32 * i + 32].bitcast(F32) for i in range(3)]
        wskipR = tb[0:64, WB5:WB5 + 32].bitcast(F32)
        hm = tb[0:128, WB6:WB6 + 4].bitcast(F32)

        def mask_h1(h1r, cout):
            # zero conv1 rows at image row -1 (u=0, half0) / 100 (u=51, half1)
            AL = mybir.AluOpType
            nc.vector.tensor_scalar(out=h1r[0:cout, 0:PROW],
                                    in0=h1r[0:cout, 0:PROW],
                                    scalar1=hm[0:cout, 0:1], scalar2=None, op0=AL.mult)
            nc.vector.tensor_scalar(out=h1r[0:cout, 51 * PROW:52 * PROW],
                                    in0=h1r[0:cout, 51 * PROW:52 * PROW],
                                    scalar1=hm[0:cout, 1:2], scalar2=None, op0=AL.mult)
            p2 = 64 if cout == 64 else 32
            nc.vector.tensor_scalar(out=h1r[p2:p2 + cout, 50 * PROW:51 * PROW],
                                    in0=h1r[p2:p2 + cout, 50 * PROW:51 * PROW],
                                    scalar1=hm[0:cout, 1:2], scalar2=None, op0=AL.mult)

        def r3(ap, nrowstile):
            return ap.rearrange("p (r c) -> p r c", c=PROW)

        # (drow, dcol, K) per matmul: 3 paired (ky=0&1) + 3 single (ky=2)
        def offs(cin):
            return [(0, kx, 2 * cin) for kx in range(3)] + \
                   [(2, kx, cin) for kx in range(3)]

        def conv1(xr, wts, om, cout, h1r, poff):
            x3 = r3(xr, NROWS)
            for j in range(11):
                y0 = 5 * j
                nrow = min(5, 52 - y0)
                n = nrow * 100
                pfull = psum.tile([64, 500], F32, tag="pconv")
                p = pfull[0:cout, :]
                for i, (dr, dc, k) in enumerate(om):
                    rhs = x3[0:k, y0 + dr:y0 + dr + nrow, dc:dc + 100]
                    nc.tensor.matmul(p[:, 0:n], wts[i], rhs,
                                     start=(i == 0), stop=(i == len(om) - 1))
                ps = p[:, 0:n].rearrange("p (r c) -> p r c", c=100)
                h3a = r3(h1r[0:cout, :], 52)
                nc.scalar.activation(h3a[:, y0:y0 + nrow, 1:101], ps, AF.Relu)
                h3b = r3(h1r[poff:poff + cout, :], 52)
                if j == 0:
                    ps1 = p[:, 100:n].rearrange("p (r c) -> p r c", c=100)
                    nc.scalar.activation(h3b[:, 0:nrow - 1, 1:101], ps1, AF.Relu)
                else:
                    nc.scalar.activation(h3b[:, y0 - 1:y0 - 1 + nrow, 1:101], ps, AF.Relu)

        def conv2(h1r, wtom, cout, elh, ext, k2, outdst):
            wts, om = wtom
            h3 = r3(h1r, 52)
            e3 = r3(ext, NROWS)
            for j in range(10):
                z0 = 5 * j
                pfull = psum.tile([64, 500], F32, tag="pconv")
                p = pfull[0:cout, :]
                for i, (dr, dc, k) in enumerate(om):
                    rhs = h3[0:k, z0 + dr:z0 + dr + 5, dc:dc + 100]
                    nc.tensor.matmul(p[:], wts[i], rhs, start=(i == 0), stop=False)
                rhs = e3[0:k2, z0 + 2:z0 + 7, 1:101]
                nc.tensor.matmul(p[:], elh, rhs, start=False, stop=True)
                nc.scalar.copy(outdst[:, 500 * j:500 * j + 500], p[:])

        h1mR = big.tile([64, 52 * PROW], F32)
        nc.vector.memset(h1mR[:], 0.0)
        fYt = big.tile([16, 5000], F32)

        m2om = (wm2, [(0, kx, 64) for kx in range(3)] + [(2, kx, 16) for kx in range(3)])
        conv1(xmR, wm1, offs(64), 16, h1mR, 32)
        mask_h1(h1mR, 16)
        conv2(h1mR, m2om, 16, wskipR, xmR, 64, fYt)

        codesT = big.tile([128, 1280], U16)
        nc.vector.memset(codesT[:], 0)
        outT = big.tile([128, 480], U8)
        nc.vector.memset(outT[:], 0)
        for blk in range(40):
            m = min(128, 5000 - blk * 128)
            pr = pidx.tile([128, 512], F32, tag="rv")
            nc.tensor.matmul(pr[0:m, :], fYt[:, blk * 128:blk * 128 + m],
                             rot_t, start=True, stop=True)
            rvsb = work.tile([128, 512], F32, tag="rvsb")
            nc.vector.tensor_copy(rvsb[0:m, :], pr[0:m, :])
            for h in range(4):
                col = blk * 4 + h
                mx = work.tile([128, 8], F32, tag="mx")
                nc.vector.max(mx[0:m, :], rvsb[0:m, h * 128:(h + 1) * 128])
                nc.vector.max_index(codesT[0:m, col * 8:col * 8 + 8],
                                    mx[0:m, :], rvsb[0:m, h * 128:(h + 1) * 128])
                nc.vector.tensor_sub(
                    outT[0:m, 160 + 2 * col:162 + 2 * col].bitcast(F16),
                    mx[0:m, 0:1], mx[0:m, 1:2])
        nc.vector.tensor_copy(outT[:, 0:160].rearrange("p (c e) -> p c e", e=1),
                              codesT[:].rearrange("p (c e) -> p c e", e=8)[:, :, 0:1])
        nc.sync.dma_start(out=outO[:], in_=outT[:])

    legalize_single_wait(nc)
    return nc


# ---- host-side input prep ----
def _pad_half_i16(q_bchw, b, r0):
    C = q_bchw.shape[1]
    out = np.zeros((C, NROWS, PROW), np.int16)
    lo, hi = r0 - 2, r0 + 53
    src_lo, src_hi = max(lo, 0), min(hi, 100)
    out[:, src_lo - lo:src_hi - lo, 1:101] = q_bchw[b, :, src_lo:src_hi, :]
    return out


def _rotpm_table(rot):
    cols = []
    for h in range(N_HASHES):
        cols.append(rot[:, h, :])
        cols.append(-rot[:, h, :])
    return np.ascontiguousarray(np.concatenate(cols, axis=1).astype(np.float32))


def make_l1_inputs(inp, rot):
    """Build the 8 per-core input dicts from the problem inputs."""
    rotpm = _rotpm_table(rot)

    def wpack(w):
        p = np.stack([np.concatenate([w[:, :, 0, kx].T, w[:, :, 1, kx].T], axis=0)
                      for kx in range(3)]).astype(np.float32)
        s = np.stack([np.ascontiguousarray(w[:, :, 2, kx].T)
                      for kx in range(3)]).astype(np.float32)
        return p, s

    def wpack_gap(w):
        p = []
        for kx in range(3):
            m = np.zeros((64, 16), np.float32)
            m[0:16] = w[:, :, 0, kx].T
            m[32:48] = w[:, :, 1, kx].T
            p.append(m)
        s = np.stack([np.ascontiguousarray(w[:, :, 2, kx].T)
                      for kx in range(3)]).astype(np.float32)
        return np.stack(p), s

    m1p, m1s = wpack(inp['mw1'])
    m2p, m2s = wpack_gap(inp['mw2'])
    wskip = np.ascontiguousarray(inp['mws'][:, :, 0, 0].T).astype(np.float32)

    tmpl = np.zeros((128, NBLOB), np.int16)
    cv = lambda a: np.ascontiguousarray(a).view(np.int16)
    for i in range(3):
        tmpl[:, WB1 + 32 * i:WB1 + 32 * i + 32] = cv(m1p[i])
        tmpl[0:64, WB2 + 32 * i:WB2 + 32 * i + 32] = cv(m1s[i])
        tmpl[0:64, WB3 + 32 * i:WB3 + 32 * i + 32] = cv(m2p[i])
        tmpl[0:16, WB4 + 32 * i:WB4 + 32 * i + 32] = cv(m2s[i])
    tmpl[0:64, WB5:WB5 + 32] = cv(wskip)
    tmpl[:, WB7:WB7 + 128] = rotpm.view(np.int16).reshape(128, 128)
    hms = [np.broadcast_to(np.array([1.0 if half == 1 else 0.0,
                                     1.0 if half == 0 else 0.0], np.float32),
                           (128, 2)).copy().view(np.int16) for half in range(2)]

    q1 = np.rint(inp['feature_dec1'] * (1.0 / SCALE)).astype(np.int16)
    q2 = np.rint(inp['feature_dec2'] * (1.0 / SCALE)).astype(np.int16)
    m_units = [(q1, 0), (q2, 0), (q1, 1), (q2, 1)]

    in_maps = []
    for c in range(8):
        msrc, mb = m_units[c // 2]
        half = c % 2
        bl = tmpl.copy()
        img = _pad_half_i16(msrc, mb, half * 50).reshape(64, -1)
        bl[0:64, 0:IMGC] = img[:, 0:IMGC]
        bl[64:128, 0:IMGC] = img[:, IMGC:2 * IMGC]
        bl[:, WB6:WB6 + 4] = hms[half]
        in_maps.append({'blob': bl})
    return in_maps


def _run_cached(nc, in_maps, n_cores=8):
    """run_bass_via_pjrt with the jitted executable memoized, so warm calls
    skip re-trace/lowering of the PJRT program."""
    import jax
    from jax.sharding import Mesh, PartitionSpec
    from jax.experimental.shard_map import shard_map
    from concourse import bass2jax, mybir as _mb

    if "exec" not in _CACHE:
        bass2jax.install_neuronx_cc_hook()
        part_name = nc.partition_id_tensor.name if nc.partition_id_tensor else None
        in_names, out_names, out_avals, zero_shapes = [], [], [], []
        for alloc in nc.m.functions[0].allocations:
            if not isinstance(alloc, _mb.MemoryLocationSet):
                continue
            name = alloc.memorylocations[0].name
            if alloc.kind == "ExternalInput":
                if name != part_name:
                    in_names.append(name)
            elif alloc.kind == "ExternalOutput":
                out_names.append(name)
                shape = tuple(alloc.tensor_shape)
                dtype = _mb.dt.np(alloc.dtype)
                out_avals.append(jax.core.ShapedArray(shape, dtype))
                zero_shapes.append((shape, dtype))
        n_params = len(in_names)
        bind_names = tuple(in_names + out_names + ([part_name] if part_name else []))

        def _body(*args):
            operands = list(args)
            if part_name:
                operands.append(bass2jax.partition_id_tensor())
            outs = bass2jax._bass_exec_p.bind(
                *operands, out_avals=tuple(out_avals), in_names=bind_names,
                out_names=tuple(out_names), lowering_input_output_aliases=(),
                sim_require_finite=True, sim_require_nnan=True, nc=nc)
            return tuple(outs)

        devices = jax.devices()[:n_cores]
        mesh = Mesh(np.asarray(devices), ("core",))
        n_out = len(out_names)
        sharded = jax.jit(
            shard_map(_body, mesh=mesh,
                      in_specs=(PartitionSpec("core"),) * (n_params + n_out),
                      out_specs=(PartitionSpec("core"),) * n_out,
                      check_rep=False),
            donate_argnums=tuple(range(n_params, n_params + n_out)),
            keep_unused=True)
        _CACHE["exec"] = (sharded, in_names, out_names, out_avals, zero_shapes)

    sharded, in_names, out_names, out_avals, zero_shapes = _CACHE["exec"]
    concat_in = [np.concatenate([np.asarray(m[name]) for m in in_maps], axis=0)
                 for name in in_names]
    concat_zeros = [np.zeros((n_cores * s[0], *s[1:]), d) for s, d in zero_shapes]
    out_arrs = sharded(*concat_in, *concat_zeros)
    return [
        {name: np.asarray(out_arrs[i]).reshape(n_cores, *out_avals[i].shape)[c]
         for i, name in enumerate(out_names)}
        for c in range(n_cores)
    ]


def _get_nc():
    if "nc" not in _CACHE:
        _CACHE["nc"] = build_l1()
    return _CACHE["nc"]


# ---- host-side exact pixel resblock (vectorized over positions) ----
def respix_batch(x_chw, pts, w1, b1, w2, b2, ws=None, bs=None):
    """Exact resblock outputs at flat pixel indices pts of one image.
    x_chw: (Ci,100,100) f32; returns (P, Co) f32."""
    pts = np.asarray(pts, np.int64)
    P = len(pts)
    y, xc = pts // 100, pts % 100
    Ci = x_chw.shape[0]
    xp = np.zeros((Ci, 104, 104), np.float32)
    xp[:, 2:102, 2:102] = x_chw
    rows = y[:, None] + np.arange(5)[None, :]
    cols = xc[:, None] + np.arange(5)[None, :]
    patches = xp[:, rows[:, :, None], cols[:, None, :]]       # (Ci,P,5,5)
    win = sliding_window_view(patches, (3, 3), axis=(2, 3))   # (Ci,P,3,3,3,3)
    xm = win.transpose(1, 2, 3, 0, 4, 5).reshape(P * 9, Ci * 9)
    W1 = w1.reshape(w1.shape[0], Ci * 9).astype(np.float32)
    h = (xm @ W1.T).reshape(P, 3, 3, -1) + b1.astype(np.float32)
    uval = ((y[:, None] - 1 + np.arange(3)[None, :]) >= 0) & \
           ((y[:, None] - 1 + np.arange(3)[None, :]) < 100)
    vval = ((xc[:, None] - 1 + np.arange(3)[None, :]) >= 0) & \
           ((xc[:, None] - 1 + np.arange(3)[None, :]) < 100)
    h = h * uval[:, :, None, None] * vval[:, None, :, None]
    h = np.maximum(h, 0)
    W2 = w2.transpose(0, 2, 3, 1).reshape(w2.shape[0], -1).astype(np.float32)
    out = h.reshape(P, -1) @ W2.T + b2.astype(np.float32)
    x_c = x_chw[:, y, xc].T                                   # (P,Ci)
    if ws is None:
        out = out + x_c
    else:
        out = out + x_c @ ws[:, :, 0, 0].T.astype(np.float32) + bs.astype(np.float32)
    return out


def _q_at(inp, b, t):
    """Exact m-resblock embedding (16,) at f1f2 position t of batch b."""
    src = inp['feature_dec1'] if t < L else inp['feature_dec2']
    return respix_batch(src[b], [t % L], inp['mw1'], inp['mb1'],
                        inp['mw2'], inp['mb2'], inp['mws'], inp['mbs'])[0]


def _r_at(inp, b, t):
    """Exact [aux1|refe] embedding (64,) at concat position t of batch b."""
    if t < L:
        return respix_batch(inp['feature_dec1'][b], [t], inp['a1w1'], inp['a1b1'],
                            inp['a1w2'], inp['a1b2'])[0]
    return respix_batch(inp['reference_feature'][b], [t - L], inp['a2w1'],
                        inp['a2b1'], inp['a2w2'], inp['a2b2'])[0]


def kernel(**inputs):
    import jax, time
    inp = {k: np.asarray(v) for k, v in inputs.items()}
    ri = inp["random_index"].astype(np.int64)
    rot = np.asarray(jax.random.normal(jax.random.key(42), (16, N_HASHES, HB // 2),
                                       dtype=jax.numpy.float32))
    nc = _get_nc()
    in_maps = make_l1_inputs(inp, rot)
    _t0 = time.time()
    try:
        res = _run_cached(nc, in_maps)
    except Exception:
        _CACHE.pop("exec", None)
        from concourse.bass_utils import run_bass_kernel_spmd
        res = run_bass_kernel_spmd(nc, in_maps, list(range(8))).results
    _CACHE["device_wall_ns"] = int((time.time() - _t0) * 1e9)

    codes = np.zeros((2, N_HASHES, 2 * L), np.int64)
    gaps = np.ones((2, N_HASHES, 2 * L), np.float32)
    for c in range(8):
        b, q = c // 4, c % 4
        ro = res[c]["outO"]
        cu = ro[:, 0:160]
        gu = np.ascontiguousarray(ro[:, 160:480]).view(np.float16).astype(np.float32)
        for h in range(N_HASHES):
            codes[b, h, q * 5000:(q + 1) * 5000] = \
                cu[:, h::4].astype(np.int64).T.reshape(-1)[:5000]
            gaps[b, h, q * 5000:(q + 1) * 5000] = gu[:, h::4].T.reshape(-1)[:5000]

    # exact fixup of near-tie argmaxes (quantization could have flipped them)
    for b in range(2):
        sus = gaps[b] < TAU                       # (nh, 2L)
        pos = np.where(sus.any(axis=0))[0]
        if len(pos) == 0:
            continue
        f_ex = np.zeros((len(pos), 16), np.float32)
        lo = pos < L
        if lo.any():
            f_ex[lo] = respix_batch(inp['feature_dec1'][b], pos[lo],
                                    inp['mw1'], inp['mb1'], inp['mw2'], inp['mb2'],
                                    inp['mws'], inp['mbs'])
        if (~lo).any():
            f_ex[~lo] = respix_batch(inp['feature_dec2'][b], pos[~lo] - L,
                                     inp['mw1'], inp['mb1'], inp['mw2'], inp['mb2'],
                                     inp['mws'], inp['mbs'])
        for h in range(N_HASHES):
            hs = np.where(sus[h])[0]
            if len(hs) == 0:
                continue
            fv = f_ex[np.searchsorted(pos, hs)]
            rv = fv @ rot[:, h, :]
            codes[b, h, hs] = np.argmax(np.concatenate([rv, -rv], axis=1), axis=1)

    # rank-2 combine: out = w*rA + (1-w)*rB per position
    out = np.zeros((2, 64, L), np.float32)
    tt = np.arange(2 * L)
    X = (tt % 2).astype(np.int64)
    jA, jB = int(ri[0]), int(ri[L])
    padk = CHUNK - (2 * L) % CHUNK
    kch = (2 * L + padk) // CHUNK
    for b in range(2):
        qA, qB = _q_at(inp, b, jA).astype(np.float64), _q_at(inp, b, jB).astype(np.float64)
        zA = 0.01 if jA < L else 0.99
        zB = 0.01 if jB < L else 0.99
        rA, rB = _r_at(inp, b, jA), _r_at(inp, b, jB)
        nh = lambda v: v / max(np.sqrt(np.sum(v ** 2)), 5e-5)
        Ah, Bh = nh(qA), nh(qB)
        s = np.array([[qA @ Ah, qA @ Bh], [qB @ Ah, qB @ Bh]])
        Asum = np.zeros(2 * L, np.float64)
        for h in range(N_HASHES):
            cp = codes[b, h][ri]
            order = np.argsort(cp, kind="stable")
            slot = np.empty(2 * L, np.int64)
            slot[order] = tt
            chunk = slot // CHUNK
            ev = X == 0
            na = np.bincount(chunk[ev], minlength=kch).astype(np.int64)
            na[kch - 1] += np.count_nonzero((slot >= 2 * L - padk) & ev)
            na3 = na + np.roll(na, 1) + np.roll(na, -1)
            Asum += na3[chunk]
        keep = ri < L
        eA = np.exp(s[:, 0])[X[keep]] * zA
        eB = np.exp(s[:, 1])[X[keep]] * zB
        u = Asum[keep] * eA
        v = (N_HASHES * 3 * CHUNK - Asum[keep]) * eB
        w = (u / (u + v)).astype(np.float32)
        comb = w[:, None] * rA[None, :].astype(np.float32) + \
               (1 - w)[:, None] * rB[None, :].astype(np.float32)
        out[b][:, ri[keep]] = comb.T
    return out.reshape(2, 64, 100, 100)


# revision 28
# speedup vs baseline: 7.9420x; 1.0602x over previous
"""Trainium2 kernel for nn_Non_Local_Sparse_Network (sparse_attention).

The attention algebra collapses to rank-2: mod_indices = (indices % 2) * L
means every query/value row of the bucketed attention is one of just two
vectors (rows ri[0] and ri[L] of the permuted embeddings).  So the final
output is w*rA + (1-w)*rB per position, where only the LSH bucket codes
(from the 16-channel m-resblock embeddings) require full-image compute.

Device (8 NeuronCores, SPMD, one (batch, f1/f2, half) m-unit per core):
int16-dequant + the m-resblock convs in exact f32 (TensorE), the LSH
rv = f @ [rot|-rot] matmul, per-hash argmax codes (DVE max/max_index),
plus the top1-top2 gap so the host can exactly re-resolve near-ties.
Host: 5x5-patch resblocks for the four distinguished pixels (qA/qB/rA/rB),
gap-thresholded exact fixup of bucket codes, and the O(N) counting-sort
combine.  Only ~6.5MB up / ~0.5MB down crosses the device link.
"""
import numpy as np

"""Patch TileContext._drain_and_barrier: this walrus build only accepts one
sync-wait on an SP Drain, so split the tail drain's waits across a chain of
single-wait drains."""
import bass_rust
import concourse.tile as _tile
from concourse.vector_clock import ScopedClock


def _drain_and_barrier_split(self, tick_clock, wait_clock):
    drain_inst = self.nc.sync.drain()
    wait_clock.add_sem_waits(
        drain_inst.ins, ScopedClock({None: tick_clock.global_clock})
    )
    si = drain_inst.ins.sync_info
    waits = list(si.on_wait)
    if len(waits) > 1:
        drain_inst.ins.sync_info = bass_rust.SyncInfo(
            on_wait=[waits[0]], on_update=list(si.on_update)
        )
        for w in waits[1:]:
            extra = self.nc.sync.drain()
            extra.ins.sync_info = bass_rust.SyncInfo(on_wait=[w], on_update=[])

    self.nc.all_engine_barrier()
    assert self.sems is not None
    popped = self.nc._tile_sem_poison_stack.pop()
    assert popped is self._sem_poison
    self.nc.clear_and_free_semaphores(list(self.sems.allocated().values()))
    self.nc.all_engine_barrier()


_tile.TileContext._drain_and_barrier = _drain_and_barrier_split


def legalize_single_wait(nc):
    """This walrus build allows at most one sync-wait per instruction.
    For any instruction carrying k>1 waits, hoist k-1 of them onto fresh
    same-engine NOPs inserted immediately before it (same-engine program
    order makes this semantically identical)."""
    import concourse.mybir as mybir

    def make_nop(engine_type):
        eng = nc.engines[engine_type]
        binst = eng.nop()
        ins = binst.ins
        # eng.nop() appended to the current bb; pull it back out
        for fn in nc.m.functions:
            for bb in fn.blocks:
                il = bb.instructions
                if il and il[-1] is ins:
                    del il[-1]
                    return ins
        raise RuntimeError("fresh nop not found at tail of any bb")

    n_fixed = 0
    for fn in nc.m.functions:
        for bb in fn.blocks:
            il = bb.instructions
            i = 0
            while i < len(il):
                inst = il[i]
                try:
                    si = inst.sync_info
                except Exception:
                    si = None
                if si is None:
                    i += 1
                    continue
                waits = list(si.on_wait)
                if len(waits) > 1:
                    for w in waits[:-1]:
                        nop = make_nop(inst.engine)
                        nop.sync_info = bass_rust.SyncInfo(on_wait=[w], on_update=[])
                        il.insert(i, nop)
                        i += 1
                    inst.sync_info = bass_rust.SyncInfo(
                        on_wait=[waits[-1]], on_update=list(si.on_update)
                    )
                    n_fixed += 1
                i += 1
    return n_fixed


import concourse.bass as bass
import concourse.mybir as mybir
import concourse.tile as tile
from contextlib import ExitStack
from numpy.lib.stride_tricks import sliding_window_view

F32 = mybir.dt.float32
F16 = mybir.dt.float16
I16 = mybir.dt.int16
U16 = mybir.dt.uint16
U8 = mybir.dt.uint8
PROW = 102
NROWS = 55
NFLAT = NROWS * PROW  # 5610
AF = mybir.ActivationFunctionType

N_HASHES, CHUNK, L, HB = 4, 144, 10000, 128
SCALE = 6.0 / 32767.0
TAU = 5e-3  # gap threshold for host-side exact re-resolution of codes
_CACHE = {}

# single-input blob layout (int16 columns; f32 payloads bitcast to i16 pairs)
IMGC = 2805            # half of NFLAT: image ships as [128, 2805]
WB1 = 2806             # wm1p: 3 x [128,16] f32
WB2 = WB1 + 96         # wm1s: 3 x [64,16] f32
WB3 = WB2 + 96         # wm2p: 3 x [64,16] f32 (gap layout)
WB4 = WB3 + 96         # wm2s: 3 x [16,16] f32
WB5 = WB4 + 96         # wskip: [64,16] f32
WB6 = WB5 + 32         # hmask: [128,2] f32
WB7 = WB6 + 4          # rotpm: [16,512] f32 as [128,128] i16
NBLOB = WB7 + 128


def build_l1():
    nc = bass.Bass("TRN2", target_bir_lowering=False, debug=False, num_devices=8)
    blob = nc.dram_tensor("blob", [128, NBLOB], I16, kind="ExternalInput")
    outO = nc.dram_tensor("outO", [128, 160], U8, kind="ExternalOutput")

    with tile.TileContext(nc) as tc, ExitStack() as ctx:
        big = ctx.enter_context(tc.tile_pool(name="big", bufs=1))
        work = ctx.enter_context(tc.tile_pool(name="work", bufs=3))
        psum = ctx.enter_context(tc.tile_pool(name="psum", bufs=2, space="PSUM"))
        pidx = ctx.enter_context(tc.tile_pool(name="pidx", bufs=2, space="PSUM"))

        tb = big.tile([128, NBLOB], I16)
        nc.sync.dma_start(out=tb[:], in_=blob[:])
        # rot table re-read from DRAM into its [16,512] layout
        rt16 = big.tile([16, 1024], I16)
        nc.sync.dma_start(out=rt16[:].rearrange("p (a b) -> p a b", b=128),
                          in_=blob[:, WB7:WB7 + 128].rearrange("(q a) c -> q a c", a=8))
        rot_t = rt16[:].bitcast(F32)

        # dequant int16 -> f32 into the row-paired layout (rows 64:128 are
        # the image shifted down one row, so conv ky=0,1 fuse into one matmul)
        xmR = big.tile([128, NFLAT], F32)
        nc.scalar.mul(xmR[0:64, 0:IMGC], tb[0:64, 0:IMGC], SCALE)
        nc.scalar.mul(xmR[0:64, IMGC:2 * IMGC], tb[64:128, 0:IMGC], SCALE)
        nc.scalar.mul(xmR[64:128, 0:IMGC - PROW], tb[0:64, PROW:IMGC], SCALE)
        nc.scalar.mul(xmR[64:128, IMGC - PROW:2 * IMGC - PROW], tb[64:128, 0:IMGC], SCALE)

        wm1 = [tb[0:128, WB1 + 32 * i:WB1 + 32 * i + 32].bitcast(F32) for i in range(3)] + \
              [tb[0:64, WB2 + 32 * i:WB2 + 32 * i + 32].bitcast(F32) for i in range(3)]
        wm2 = [tb[0:64, WB3 + 32 * i:WB3 + 32 * i + 32].bitcast(F32) for i in range(3)] + \
              [tb[0:16, WB4 + 32 * i:WB4 + 32 * i + 32].bitcast(F32) for i in range(3)]
        wskipR = tb[0:64, WB5:WB5 + 32].bitcast(F32)
        hm = tb[0:128, WB6:WB6 + 4].bitcast(F32)

        def mask_h1(h1r, cout):
            # zero conv1 rows at image row -1 (u=0, half0) / 100 (u=51, half1)
            AL = mybir.AluOpType
            nc.vector.tensor_scalar(out=h1r[0:cout, 0:PROW],
                                    in0=h1r[0:cout, 0:PROW],
                                    scalar1=hm[0:cout, 0:1], scalar2=None, op0=AL.mult)
            nc.vector.tensor_scalar(out=h1r[0:cout, 51 * PROW:52 * PROW],
                                    in0=h1r[0:cout, 51 * PROW:52 * PROW],
                                    scalar1=hm[0:cout, 1:2], scalar2=None, op0=AL.mult)
            p2 = 64 if cout == 64 else 32
            nc.vector.tensor_scalar(out=h1r[p2:p2 + cout, 50 * PROW:51 * PROW],
                                    in0=h1r[p2:p2 + cout, 50 * PROW:51 * PROW],
                                    scalar1=hm[0:cout, 1:2], scalar2=None, op0=AL.mult)

        def r3(ap, nrowstile):
            return ap.rearrange("p (r c) -> p r c", c=PROW)

        # (drow, dcol, K) per matmul: 3 paired (ky=0&1) + 3 single (ky=2)
        def offs(cin):
            return [(0, kx, 2 * cin) for kx in range(3)] + \
                   [(2, kx, cin) for kx in range(3)]

        def conv1(xr, wts, om, cout, h1r, poff):
            x3 = r3(xr, NROWS)
            for j in range(11):
                y0 = 5 * j
                nrow = min(5, 52 - y0)
                n = nrow * 100
                pfull = psum.tile([64, 500], F32, tag="pconv")
                p = pfull[0:cout, :]
                for i, (dr, dc, k) in enumerate(om):
                    rhs = x3[0:k, y0 + dr:y0 + dr + nrow, dc:dc + 100]
                    nc.tensor.matmul(p[:, 0:n], wts[i], rhs,
                                     start=(i == 0), stop=(i == len(om) - 1))
                ps = p[:, 0:n].rearrange("p (r c) -> p r c", c=100)
                h3a = r3(h1r[0:cout, :], 52)
                nc.scalar.activation(h3a[:, y0:y0 + nrow, 1:101], ps, AF.Relu)
                h3b = r3(h1r[poff:poff + cout, :], 52)
                if j == 0:
                    ps1 = p[:, 100:n].rearrange("p (r c) -> p r c", c=100)
                    nc.scalar.activation(h3b[:, 0:nrow - 1, 1:101], ps1, AF.Relu)
                else:
                    nc.scalar.activation(h3b[:, y0 - 1:y0 - 1 + nrow, 1:101], ps, AF.Relu)

        def conv2(h1r, wtom, cout, elh, ext, k2, outdst):
            wts, om = wtom
            h3 = r3(h1r, 52)
            e3 = r3(ext, NROWS)
            for j in range(10):
                z0 = 5 * j
                pfull = psum.tile([64, 500], F32, tag="pconv")
                p = pfull[0:cout, :]
                for i, (dr, dc, k) in enumerate(om):
                    rhs = h3[0:k, z0 + dr:z0 + dr + 5, dc:dc + 100]
                    nc.tensor.matmul(p[:], wts[i], rhs, start=(i == 0), stop=False)
                rhs = e3[0:k2, z0 + 2:z0 + 7, 1:101]
                nc.tensor.matmul(p[:], elh, rhs, start=False, stop=True)
                nc.scalar.copy(outdst[:, 500 * j:500 * j + 500], p[:])

        h1mR = big.tile([64, 52 * PROW], F32)
        nc.vector.memset(h1mR[:], 0.0)
        fYt = big.tile([16, 5000], F32)

        m2om = (wm2, [(0, kx, 64) for kx in range(3)] + [(2, kx, 16) for kx in range(3)])
        conv1(xmR, wm1, offs(64), 16, h1mR, 32)
        mask_h1(h1mR, 16)
        conv2(h1mR, m2om, 16, wskipR, xmR, 64, fYt)

        AL = mybir.AluOpType
        codesT = big.tile([128, 1280], U16)
        nc.vector.memset(codesT[:], 0)
        gapF = big.tile([128, 160], F32)
        nc.vector.memset(gapF[:], 0.0)
        for blk in range(40):
            m = min(128, 5000 - blk * 128)
            pr = pidx.tile([128, 512], F32, tag="rv")
            nc.tensor.matmul(pr[0:m, :], fYt[:, blk * 128:blk * 128 + m],
                             rot_t, start=True, stop=True)
            rvsb = work.tile([128, 512], F32, tag="rvsb")
            nc.vector.tensor_copy(rvsb[0:m, :], pr[0:m, :])
            for h in range(4):
                col = blk * 4 + h
                mx = work.tile([128, 8], F32, tag="mx")
                nc.vector.max(mx[0:m, :], rvsb[0:m, h * 128:(h + 1) * 128])
                nc.vector.max_index(codesT[0:m, col * 8:col * 8 + 8],
                                    mx[0:m, :], rvsb[0:m, h * 128:(h + 1) * 128])
                nc.vector.tensor_sub(gapF[0:m, col:col + 1],
                                     mx[0:m, 0:1], mx[0:m, 1:2])
        # pack: bit7 = near-tie flag (host re-resolves exactly), bits 0:7 = code
        flags = big.tile([128, 160], U8)
        nc.vector.tensor_scalar(out=flags[:], in0=gapF[:], scalar1=TAU,
                                scalar2=None, op0=AL.is_lt)
        nc.vector.tensor_scalar(out=flags[:], in0=flags[:], scalar1=128,
                                scalar2=None, op0=AL.mult)
        outT = big.tile([128, 160], U8)
        nc.vector.tensor_copy(outT[:].rearrange("p (c e) -> p c e", e=1),
                              codesT[:].rearrange("p (c e) -> p c e", e=8)[:, :, 0:1])
        nc.vector.tensor_add(outT[:], outT[:], flags[:])
        nc.sync.dma_start(out=outO[:], in_=outT[:])

    legalize_single_wait(nc)
    return nc


# ---- host-side input prep ----
def _pad_half_i16(q_bchw, b, r0):
    C = q_bchw.shape[1]
    out = np.zeros((C, NROWS, PROW), np.int16)
    lo, hi = r0 - 2, r0 + 53
    src_lo, src_hi = max(lo, 0), min(hi, 100)
    out[:, src_lo - lo:src_hi - lo, 1:101] = q_bchw[b, :, src_lo:src_hi, :]
    return out


def _rotpm_table(rot):
    cols = []
    for h in range(N_HASHES):
        cols.append(rot[:, h, :])
        cols.append(-rot[:, h, :])
    return np.ascontiguousarray(np.concatenate(cols, axis=1).astype(np.float32))


def make_l1_inputs(inp, rot):
    """Build the 8 per-core input dicts from the problem inputs."""
    rotpm = _rotpm_table(rot)

    def wpack(w):
        p = np.stack([np.concatenate([w[:, :, 0, kx].T, w[:, :, 1, kx].T], axis=0)
                      for kx in range(3)]).astype(np.float32)
        s = np.stack([np.ascontiguousarray(w[:, :, 2, kx].T)
                      for kx in range(3)]).astype(np.float32)
        return p, s

    def wpack_gap(w):
        p = []
        for kx in range(3):
            m = np.zeros((64, 16), np.float32)
            m[0:16] = w[:, :, 0, kx].T
            m[32:48] = w[:, :, 1, kx].T
            p.append(m)
        s = np.stack([np.ascontiguousarray(w[:, :, 2, kx].T)
                      for kx in range(3)]).astype(np.float32)
        return np.stack(p), s

    m1p, m1s = wpack(inp['mw1'])
    m2p, m2s = wpack_gap(inp['mw2'])
    wskip = np.ascontiguousarray(inp['mws'][:, :, 0, 0].T).astype(np.float32)

    tmpl = np.zeros((128, NBLOB), np.int16)
    cv = lambda a: np.ascontiguousarray(a).view(np.int16)
    for i in range(3):
        tmpl[:, WB1 + 32 * i:WB1 + 32 * i + 32] = cv(m1p[i])
        tmpl[0:64, WB2 + 32 * i:WB2 + 32 * i + 32] = cv(m1s[i])
        tmpl[0:64, WB3 + 32 * i:WB3 + 32 * i + 32] = cv(m2p[i])
        tmpl[0:16, WB4 + 32 * i:WB4 + 32 * i + 32] = cv(m2s[i])
    tmpl[0:64, WB5:WB5 + 32] = cv(wskip)
    tmpl[:, WB7:WB7 + 128] = rotpm.view(np.int16).reshape(128, 128)
    hms = [np.broadcast_to(np.array([1.0 if half == 1 else 0.0,
                                     1.0 if half == 0 else 0.0], np.float32),
                           (128, 2)).copy().view(np.int16) for half in range(2)]

    q1 = np.rint(inp['feature_dec1'] * (1.0 / SCALE)).astype(np.int16)
    q2 = np.rint(inp['feature_dec2'] * (1.0 / SCALE)).astype(np.int16)
    m_units = [(q1, 0), (q2, 0), (q1, 1), (q2, 1)]

    blob_all = np.empty((8 * 128, NBLOB), np.int16)
    for c in range(8):
        msrc, mb = m_units[c // 2]
        half = c % 2
        bl = blob_all[c * 128:(c + 1) * 128]
        bl[:] = tmpl
        img = _pad_half_i16(msrc, mb, half * 50).reshape(64, -1)
        bl[0:64, 0:IMGC] = img[:, 0:IMGC]
        bl[64:128, 0:IMGC] = img[:, IMGC:2 * IMGC]
        bl[:, WB6:WB6 + 4] = hms[half]
    return blob_all


def _run_cached(nc, blob_all, n_cores=8):
    """run_bass_via_pjrt with the jitted executable memoized, so warm calls
    skip re-trace/lowering of the PJRT program."""
    import jax
    from jax.sharding import Mesh, PartitionSpec
    from jax.experimental.shard_map import shard_map
    from concourse import bass2jax, mybir as _mb

    if "exec" not in _CACHE:
        bass2jax.install_neuronx_cc_hook()
        part_name = nc.partition_id_tensor.name if nc.partition_id_tensor else None
        in_names, out_names, out_avals, zero_shapes = [], [], [], []
        for alloc in nc.m.functions[0].allocations:
            if not isinstance(alloc, _mb.MemoryLocationSet):
                continue
            name = alloc.memorylocations[0].name
            if alloc.kind == "ExternalInput":
                if name != part_name:
                    in_names.append(name)
            elif alloc.kind == "ExternalOutput":
                out_names.append(name)
                shape = tuple(alloc.tensor_shape)
                dtype = _mb.dt.np(alloc.dtype)
                out_avals.append(jax.core.ShapedArray(shape, dtype))
                zero_shapes.append((shape, dtype))
        n_params = len(in_names)
        bind_names = tuple(in_names + out_names + ([part_name] if part_name else []))

        def _body(*args):
            operands = list(args)
            if part_name:
                operands.append(bass2jax.partition_id_tensor())
            outs = bass2jax._bass_exec_p.bind(
                *operands, out_avals=tuple(out_avals), in_names=bind_names,
                out_names=tuple(out_names), lowering_input_output_aliases=(),
                sim_require_finite=True, sim_require_nnan=True, nc=nc)
            return tuple(outs)

        devices = jax.devices()[:n_cores]
        mesh = Mesh(np.asarray(devices), ("core",))
        n_out = len(out_names)
        sharded = jax.jit(
            shard_map(_body, mesh=mesh,
                      in_specs=(PartitionSpec("core"),) * (n_params + n_out),
                      out_specs=(PartitionSpec("core"),) * n_out,
                      check_rep=False),
            donate_argnums=tuple(range(n_params, n_params + n_out)),
            keep_unused=True)
        _CACHE["exec"] = (sharded, in_names, out_names, out_avals, zero_shapes)

    sharded, in_names, out_names, out_avals, zero_shapes = _CACHE["exec"]
    assert in_names == ["blob"]
    concat_zeros = [np.zeros((n_cores * s[0], *s[1:]), d) for s, d in zero_shapes]
    out_arrs = sharded(blob_all, *concat_zeros)  # async dispatch
    return out_arrs


def _collect(out_arrs, n_cores=8):
    _, _, out_names, out_avals, _ = _CACHE["exec"]
    outs_np = [np.asarray(a).reshape(n_cores, *out_avals[i].shape)
               for i, a in enumerate(out_arrs)]
    return [
        {name: outs_np[i][c] for i, name in enumerate(out_names)}
        for c in range(n_cores)
    ]


def _get_nc():
    if "nc" not in _CACHE:
        _CACHE["nc"] = build_l1()
    return _CACHE["nc"]


# ---- host-side exact pixel resblock (vectorized over positions) ----
def _padded(x_chw):
    pads = _CACHE.setdefault("pads", {})
    key = (x_chw.__array_interface__['data'][0], x_chw.shape)
    csum = float(np.sum(x_chw))
    hit = pads.get(key)
    if hit is not None and hit[2] == csum:
        return hit[1]
    Ci = x_chw.shape[0]
    xp = np.zeros((Ci, 104, 104), np.float32)
    xp[:, 2:102, 2:102] = x_chw
    if len(pads) > 16:
        pads.clear()
    pads[key] = (x_chw, xp, csum)  # hold x_chw so its data pointer stays valid
    return xp


def respix_batch(x_chw, pts, w1, b1, w2, b2, ws=None, bs=None):
    """Exact resblock outputs at flat pixel indices pts of one image.
    x_chw: (Ci,100,100) f32; returns (P, Co) f32."""
    pts = np.asarray(pts, np.int64)
    P = len(pts)
    y, xc = pts // 100, pts % 100
    Ci = x_chw.shape[0]
    if P <= 8:
        xp = np.zeros((P, Ci, 5, 5), np.float32)
        for i in range(P):
            yy, xx = int(y[i]), int(xc[i])
            y0, y1 = max(yy - 2, 0), min(yy + 3, 100)
            x0, x1 = max(xx - 2, 0), min(xx + 3, 100)
            xp[i, :, y0 - (yy - 2):y1 - (yy - 2), x0 - (xx - 2):x1 - (xx - 2)] = \
                x_chw[:, y0:y1, x0:x1]
        patches = xp.transpose(1, 0, 2, 3)                    # (Ci,P,5,5)
    else:
        xp = _padded(x_chw)
        rows = y[:, None] + np.arange(5)[None, :]
        cols = xc[:, None] + np.arange(5)[None, :]
        patches = xp[:, rows[:, :, None], cols[:, None, :]]   # (Ci,P,5,5)
    win = sliding_window_view(patches, (3, 3), axis=(2, 3))   # (Ci,P,3,3,3,3)
    xm = win.transpose(1, 2, 3, 0, 4, 5).reshape(P * 9, Ci * 9)
    W1 = w1.reshape(w1.shape[0], Ci * 9).astype(np.float32)
    h = (xm @ W1.T).reshape(P, 3, 3, -1) + b1.astype(np.float32)
    uval = ((y[:, None] - 1 + np.arange(3)[None, :]) >= 0) & \
           ((y[:, None] - 1 + np.arange(3)[None, :]) < 100)
    vval = ((xc[:, None] - 1 + np.arange(3)[None, :]) >= 0) & \
           ((xc[:, None] - 1 + np.arange(3)[None, :]) < 100)
    h = h * uval[:, :, None, None] * vval[:, None, :, None]
    h = np.maximum(h, 0)
    W2 = w2.transpose(0, 2, 3, 1).reshape(w2.shape[0], -1).astype(np.float32)
    out = h.reshape(P, -1) @ W2.T + b2.astype(np.float32)
    x_c = x_chw[:, y, xc].T                                   # (P,Ci)
    if ws is None:
        out = out + x_c
    else:
        out = out + x_c @ ws[:, :, 0, 0].T.astype(np.float32) + bs.astype(np.float32)
    return out


def _q_at(inp, b, t):
    """Exact m-resblock embedding (16,) at f1f2 position t of batch b."""
    src = inp['feature_dec1'] if t < L else inp['feature_dec2']
    return respix_batch(src[b], [t % L], inp['mw1'], inp['mb1'],
                        inp['mw2'], inp['mb2'], inp['mws'], inp['mbs'])[0]


def _r_at(inp, b, t):
    """Exact [aux1|refe] embedding (64,) at concat position t of batch b."""
    if t < L:
        return respix_batch(inp['feature_dec1'][b], [t], inp['a1w1'], inp['a1b1'],
                            inp['a1w2'], inp['a1b2'])[0]
    return respix_batch(inp['reference_feature'][b], [t - L], inp['a2w1'],
                        inp['a2b1'], inp['a2w2'], inp['a2b2'])[0]


def kernel(**inputs):
    import time
    inp = {k: np.asarray(v) for k, v in inputs.items()}
    ri = inp["random_index"].astype(np.int64)
    rot = _CACHE.get("rot")
    if rot is None:
        import jax
        with jax.default_device(jax.devices("cpu")[0]):
            rot = np.asarray(jax.random.normal(jax.random.key(42),
                                               (16, N_HASHES, HB // 2),
                                               dtype=jax.numpy.float32))
        _CACHE["rot"] = rot
    nc = _get_nc()
    blob_all = make_l1_inputs(inp, rot)
    _t0 = time.time()
    try:
        out_arrs = _run_cached(nc, blob_all)
    except Exception:
        _CACHE.pop("exec", None)
        out_arrs = None

    # overlap with the device flight: exact embeddings at the two
    # distinguished pixels, and the padded-image cache for the fixup
    jA, jB = int(ri[0]), int(ri[L])
    qr = [(_q_at(inp, b, jA).astype(np.float64),
           _q_at(inp, b, jB).astype(np.float64),
           _r_at(inp, b, jA), _r_at(inp, b, jB)) for b in range(2)]
    for b in range(2):
        _padded(inp['feature_dec1'][b])
        _padded(inp['feature_dec2'][b])

    try:
        if out_arrs is None:
            raise RuntimeError("dispatch failed")
        res = _collect(out_arrs)
    except Exception:
        _CACHE.pop("exec", None)
        from concourse.bass_utils import run_bass_kernel_spmd
        in_maps = [{'blob': blob_all[c * 128:(c + 1) * 128]} for c in range(8)]
        res = run_bass_kernel_spmd(nc, in_maps, list(range(8))).results
    _CACHE["device_wall_ns"] = int((time.time() - _t0) * 1e9)

    codes = np.zeros((2, N_HASHES, 2 * L), np.int16)
    suspect = np.zeros((2, N_HASHES, 2 * L), bool)
    for c in range(8):
        b, q = c // 4, c % 4
        cu = res[c]["outO"]                       # (128,160) u8: bit7=flag
        for h in range(N_HASHES):
            col = cu[:, h::4].T.reshape(-1)[:5000]
            codes[b, h, q * 5000:(q + 1) * 5000] = col & 127
            suspect[b, h, q * 5000:(q + 1) * 5000] = col >= 128

    # exact fixup of near-tie argmaxes (quantization could have flipped them)
    for b in range(2):
        sus = suspect[b]                          # (nh, 2L)
        pos = np.where(sus.any(axis=0))[0]
        if len(pos) == 0:
            continue
        f_ex = np.zeros((len(pos), 16), np.float32)
        lo = pos < L
        if lo.any():
            f_ex[lo] = respix_batch(inp['feature_dec1'][b], pos[lo],
                                    inp['mw1'], inp['mb1'], inp['mw2'], inp['mb2'],
                                    inp['mws'], inp['mbs'])
        if (~lo).any():
            f_ex[~lo] = respix_batch(inp['feature_dec2'][b], pos[~lo] - L,
                                     inp['mw1'], inp['mb1'], inp['mw2'], inp['mb2'],
                                     inp['mws'], inp['mbs'])
        for h in range(N_HASHES):
            hs = np.where(sus[h])[0]
            if len(hs) == 0:
                continue
            fv = f_ex[np.searchsorted(pos, hs)]
            rv = fv @ rot[:, h, :]
            codes[b, h, hs] = np.argmax(np.concatenate([rv, -rv], axis=1), axis=1)

    # rank-2 combine: out = w*rA + (1-w)*rB per position
    out = np.empty((2, 64, L), np.float32)
    tt = np.arange(2 * L)
    X = (tt % 2).astype(np.int64)
    padk = CHUNK - (2 * L) % CHUNK
    kch = (2 * L + padk) // CHUNK
    for b in range(2):
        qA, qB, rA, rB = qr[b]
        zA = 0.01 if jA < L else 0.99
        zB = 0.01 if jB < L else 0.99
        nh = lambda v: v / max(np.sqrt(np.sum(v ** 2)), 5e-5)
        Ah, Bh = nh(qA), nh(qB)
        s = np.array([[qA @ Ah, qA @ Bh], [qB @ Ah, qB @ Bh]])
        Asum = np.zeros(2 * L, np.float64)
        for h in range(N_HASHES):
            cp = codes[b, h][ri]
            order = np.argsort(cp, kind="stable")
            slot = np.empty(2 * L, np.int64)
            slot[order] = tt
            chunk = slot // CHUNK
            ev = X == 0
            na = np.bincount(chunk[ev], minlength=kch).astype(np.int64)
            na[kch - 1] += np.count_nonzero((slot >= 2 * L - padk) & ev)
            na3 = na + np.roll(na, 1) + np.roll(na, -1)
            Asum += na3[chunk]
        keep = ri < L
        eA = np.exp(s[:, 0])[X[keep]] * zA
        eB = np.exp(s[:, 1])[X[keep]] * zB
        u = Asum[keep] * eA
        v = (N_HASHES * 3 * CHUNK - Asum[keep]) * eB
        w = (u / (u + v)).astype(np.float32)
        comb = w[:, None] * rA[None, :].astype(np.float32) + \
               (1 - w)[:, None] * rB[None, :].astype(np.float32)
        out[b][:, ri[keep]] = comb.T
    return out.reshape(2, 64, 100, 100)
